# revision 8
# baseline (speedup 1.0000x reference)
"""GATv2 (2-layer, 4 heads) + linear classifier on Trainium2, 8-core SPMD.

Sharding: nodes are partitioned contiguously across 8 cores (2500 nodes/core).
Edges are routed to the core that owns their destination node, so the
segment-softmax and scatter-add stay core-local.  The only cross-core
exchange is one AllGather per GAT layer of the source-side linear transform
table, which every core then gathers rows from by edge source id (the "halo
exchange" for cut edges).

Math decomposition (per layer, per head h with C=128 channels), using
leaky_relu(z) = 0.6 z + 0.4 |z| for slope 0.2:
  score_e = att_h . leaky_relu(z) = 0.6 * att.z + 0.4 * att.|z|
with z = xl[src] + xr[dst].  The src part of the linear term, adl[src] =
(att * xl).rowsum per head, is stored as 4 extra table columns so one gather
brings both the 512 features and the sums.  The dst part (adr[dst]) is
constant within each softmax segment and cancels, so it is dropped.
Indirect gathers are row-rate-limited (~12ns/row regardless of width), so
the xr[dst] side is never gathered: each 128-dst block loads its xr rows
with one direct DMA and broadcasts them to edges on the tensor engine using
the transposed one-hot (edge -> segment) matrix.
Softmax max-subtraction is skipped (scores are in [-6, 6], exact in fp32).
Per block, one-hot matmuls accumulate the exp-weight segment sum ("denom")
and the exp-weighted feature segment sum into PSUM; the block epilogue
divides by denom, adds the output bias, applies ELU, and stores the block
transposed (feature-major) for the next layer's matmuls.
"""

import numpy as np

N_NODES = 20000
IN_CH = 55
HID = 128
HEADS = 4
D = HID * HEADS  # 512
OUT_CH = 49
NCORES = 8
PER_CORE = N_NODES // NCORES  # 2500
BLOCK = 128
NEG_SLOPE = 0.2
W_TAB = D + HEADS  # 516: features + per-head att-weighted row sums


# ---------------------------------------------------------------- host prep


def _plan_edges(src, dst, n_nodes, per_core, n_cores):
    """Route edges to (core, block) by dst; pad each block to a multiple of
    128 edge slots, uniformly across cores (SPMD program must be identical).
    """
    blocks_per_core = (per_core + BLOCK - 1) // BLOCK
    core = dst // per_core
    dst_local = dst - core * per_core
    blk = dst_local // BLOCK

    counts = np.zeros((n_cores, blocks_per_core), dtype=np.int64)
    np.add.at(counts, (core, blk), 1)
    tiles = np.maximum(1, -(-counts.max(axis=0) // 128))
    offs = np.concatenate([[0], np.cumsum(tiles)])[:-1]
    tt = int(tiles.sum())

    esrc = np.zeros((n_cores, 128, tt), dtype=np.int32)
    edst = np.zeros((n_cores, 128, tt), dtype=np.int32)
    lv = np.full((n_cores, 128, tt), -1e30, dtype=np.float32)

    order = np.lexsort((blk, core))
    src_s, dstl_s = src[order], dst_local[order]
    key = core[order] * blocks_per_core + blk[order]
    bounds = np.searchsorted(key, np.arange(n_cores * blocks_per_core + 1))
    for c in range(n_cores):
        for b in range(blocks_per_core):
            k = c * blocks_per_core + b
            lo, hi = bounds[k], bounds[k + 1]
            cnt = hi - lo
            nslots = int(tiles[b]) * 128
            s = np.zeros(nslots, dtype=np.int32)
            d_ = np.zeros(nslots, dtype=np.int32)
            v = np.full(nslots, -1e30, dtype=np.float32)
            s[:cnt] = src_s[lo:hi]
            d_[:cnt] = dstl_s[lo:hi]
            d_[cnt:] = b * BLOCK  # dummy slots: stay inside this block
            v[:cnt] = 0.0
            o = int(offs[b])
            t = int(tiles[b])
            esrc[c][:, o : o + t] = s.reshape(t, 128).T
            edst[c][:, o : o + t] = d_.reshape(t, 128).T
            lv[c][:, o : o + t] = v.reshape(t, 128).T
    return dict(
        tiles=[int(t) for t in tiles],
        offs=[int(o) for o in offs],
        tt=tt,
        blocks=blocks_per_core,
        esrc=esrc,
        edst=edst,
        logvalid=lv,
    )


def preprocess(x, edge_index, w1_l, b1_l, w1_r, b1_r, att1, bias1,
               w2_l, b2_l, w2_r, b2_r, att2, bias2, w_cls, b_cls,
               n_cores=NCORES):
    x = np.asarray(x, np.float32)
    n = x.shape[0]
    per = n // n_cores
    ei = np.asarray(edge_index).astype(np.int64)
    loops = np.arange(n, dtype=np.int64)
    src = np.concatenate([ei[0], loops])
    dst = np.concatenate([ei[1], loops])

    att1_flat = np.asarray(att1, np.float32).reshape(-1)
    att2_flat = np.asarray(att2, np.float32).reshape(-1)

    plan = _plan_edges(src, dst, n, per, n_cores)

    in_ch = x.shape[1]
    aug = lambda w, b: np.concatenate(
        [np.asarray(w, np.float32), np.asarray(b, np.float32)[None, :]], axis=0
    )
    rep = lambda v: np.broadcast_to(
        np.asarray(v, np.float32)[None, :], (128, v.shape[0])
    ).copy()

    shared = {
        "w1l_aug": aug(w1_l, b1_l),
        "w1r_aug": aug(w1_r, b1_r),
        "w2l": np.asarray(w2_l, np.float32),
        "w2r": np.asarray(w2_r, np.float32),
        "b2l_row": np.asarray(b2_l, np.float32)[None, :],
        "b2r_row": np.asarray(b2_r, np.float32)[None, :],
        "wcls": np.asarray(w_cls, np.float32),
        "bcls_row": np.asarray(b_cls, np.float32)[None, :],
        "att1_rep": rep(att1_flat),
        "att2_rep": rep(att2_flat),
        "bias1_rep": rep(np.asarray(bias1, np.float32)),
        "bias2_rep": rep(np.asarray(bias2, np.float32)),
    }
    in_maps = []
    for c in range(n_cores):
        xa = np.concatenate(
            [x[c * per : (c + 1) * per].T, np.ones((1, per), np.float32)], axis=0
        )
        m = dict(shared)
        m["x_aug"] = np.ascontiguousarray(xa)
        m["esrc"] = plan["esrc"][c]
        m["edst"] = plan["edst"][c]
        m["logvalid"] = plan["logvalid"][c]
        in_maps.append(m)
    meta = dict(
        n=n, per=per, in_ch=in_ch, tiles=plan["tiles"], offs=plan["offs"],
        tt=plan["tt"], blocks=plan["blocks"], n_cores=n_cores,
    )
    return in_maps, meta


# ---------------------------------------------------------------- device


def build_program(meta, bench=False):
    import contextlib
    import concourse.bass as bass
    import concourse.tile as tile
    import concourse.mybir as mybir
    from concourse import bacc
    from concourse.masks import make_identity

    f32 = mybir.dt.float32
    i32 = mybir.dt.int32

    n = meta["n"]
    per = meta["per"]
    in_ch = meta["in_ch"]
    tiles = meta["tiles"]
    offs = meta["offs"]
    tt = meta["tt"]
    blocks = meta["blocks"]
    n_cores = meta["n_cores"]
    perp = blocks * BLOCK  # xr tables padded to full blocks

    nt_full, nt_rem = divmod(per, 128)
    node_tiles = [(i * 128, 128) for i in range(nt_full)]
    if nt_rem:
        node_tiles.append((nt_full * 128, nt_rem))

    nc = bacc.Bacc("TRN2", target_bir_lowering=False, debug=False, num_devices=n_cores)

    # register a -1.0 const AP so scalar.add(x, -1.0) lowers on the ACT engine
    _cm1 = nc.alloc_sbuf_tensor("const-float32-neg1", [128, 1], f32)
    nc.gpsimd.memset(_cm1.ap(), -1.0)
    nc.const_aps.aps[(f32, -1.0)] = _cm1.ap()

    def din(name, shape, dt=f32):
        return nc.dram_tensor(name, shape, dt, kind="ExternalInput").ap()

    x_aug = din("x_aug", [in_ch + 1, per])
    w1l_aug = din("w1l_aug", [in_ch + 1, D])
    w1r_aug = din("w1r_aug", [in_ch + 1, D])
    w2l = din("w2l", [D, D])
    w2r = din("w2r", [D, D])
    b2l_row = din("b2l_row", [1, D])
    b2r_row = din("b2r_row", [1, D])
    wcls = din("wcls", [D, OUT_CH])
    bcls_row = din("bcls_row", [1, OUT_CH])
    att1_rep = din("att1_rep", [128, D])
    att2_rep = din("att2_rep", [128, D])
    bias1_rep = din("bias1_rep", [128, D])
    bias2_rep = din("bias2_rep", [128, D])
    esrc = din("esrc", [128, tt], i32)
    edst = din("edst", [128, tt], i32)
    logvalid = din("logvalid", [128, tt])
    kreps = din("kreps", [1, 8], i32) if bench else None

    out = nc.dram_tensor("out", [per, OUT_CH], f32, kind="ExternalOutput").ap()

    with tile.TileContext(nc) as tc:
        with (
            tc.tile_pool(name="dram", bufs=1, space="DRAM") as dram,
            tc.tile_pool(name="consts", bufs=1) as consts,
            # ---- edge-phase pools, shared by both layers
            tc.tile_pool(name="eidx", bufs=2) as eidx,
            tc.tile_pool(name="eg", bufs=8) as eg,
            tc.tile_pool(name="esm", bufs=8) as esm,
            tc.tile_pool(name="eoh", bufs=4) as eoh,
            tc.tile_pool(name="ew", bufs=4) as ew,
            tc.tile_pool(name="exr", bufs=2) as exr,
            tc.tile_pool(name="eps", bufs=2, space="PSUM") as eps,
            tc.tile_pool(name="ebr", bufs=2, space="PSUM") as ebr,
            tc.tile_pool(name="etps", bufs=2, space="PSUM") as etps,
            tc.tile_pool(name="etail", bufs=2) as etail,
        ):
            # ---------- persistent DRAM intermediates
            xl1_loc = dram.tile([per, W_TAB], f32)
            xr1_loc = dram.tile([perp, D], f32)
            xl1_full = dram.tile([n, W_TAB], f32, addr_space="Shared")
            h1T = dram.tile([D, perp], f32)
            xl2_loc = dram.tile([per, W_TAB], f32)
            xr2_loc = dram.tile([perp, D], f32)
            xl2_full = dram.tile([n, W_TAB], f32, addr_space="Shared")
            h2T = dram.tile([D, perp], f32)

            # ---------- constants in SBUF
            identity = consts.tile([128, 128], f32)
            make_identity(nc, identity[:])
            iota_f = consts.tile([128, 128], f32)
            iota_i = consts.tile([128, 128], i32)
            nc.gpsimd.iota(iota_i[:], pattern=[[1, 128]], base=0, channel_multiplier=0)
            nc.vector.tensor_copy(iota_f[:], iota_i[:])
            ones_sb = consts.tile([1, 128], f32)
            nc.vector.memset(ones_sb[:], 1.0)

            if bench:
                kt = consts.tile([1, 8], i32)
                nc.sync.dma_start(kt[:], kreps[:])
                kregs = [nc.values_load(kt[0:1, j : j + 1]) for j in range(5)]

            def seg_loop(j):
                if bench:
                    return tc.For_i(0, kregs[j], 1)
                return contextlib.nullcontext()

            w1l_sb = consts.tile([in_ch + 1, D], f32)
            nc.sync.dma_start(w1l_sb[:], w1l_aug[:])
            w1r_sb = consts.tile([in_ch + 1, D], f32)
            nc.sync.dma_start(w1r_sb[:], w1r_aug[:])
            att1_sb = consts.tile([128, D], f32)
            nc.sync.dma_start(att1_sb[:], att1_rep[:])
            att2_sb = consts.tile([128, D], f32)
            nc.sync.dma_start(att2_sb[:], att2_rep[:])
            bias1_sb = consts.tile([128, D], f32)
            nc.sync.dma_start(bias1_sb[:], bias1_rep[:])
            bias2_sb = consts.tile([128, D], f32)
            nc.sync.dma_start(bias2_sb[:], bias2_rep[:])

            w2l_ch = []
            w2r_ch = []
            wcls_ch = []
            for k in range(4):
                t1 = consts.tile([128, D], f32, name=f"w2l_{k}")
                nc.sync.dma_start(t1[:], w2l[k * 128 : (k + 1) * 128, :])
                w2l_ch.append(t1)
                t2 = consts.tile([128, D], f32, name=f"w2r_{k}")
                nc.sync.dma_start(t2[:], w2r[k * 128 : (k + 1) * 128, :])
                w2r_ch.append(t2)
                t3 = consts.tile([128, OUT_CH], f32, name=f"wcls_{k}")
                nc.sync.dma_start(t3[:], wcls[k * 128 : (k + 1) * 128, :])
                wcls_ch.append(t3)
            b2l_sb = consts.tile([1, D], f32)
            nc.sync.dma_start(b2l_sb[:], b2l_row[:])
            b2r_sb = consts.tile([1, D], f32)
            nc.sync.dma_start(b2r_sb[:], b2r_row[:])
            bcls_sb = consts.tile([1, OUT_CH], f32)
            nc.sync.dma_start(bcls_sb[:], bcls_row[:])

            # zero-fill the xr tables' padded tail rows once
            if perp > per:
                zpad = consts.tile([128, D], f32)
                nc.vector.memset(zpad[:], 0.0)
                nc.sync.dma_start(xr1_loc[per:perp, :], zpad[: perp - per, :])
                nc.sync.dma_start(xr2_loc[per:perp, :], zpad[: perp - per, :])

            # ================= stage 0: layer-1 dense transforms (local rows)
            with (
                tc.tile_pool(name="s0_in", bufs=3) as s0in,
                tc.tile_pool(name="s0_out", bufs=3) as s0out,
                seg_loop(0),
            ):
                for base, m in node_tiles:
                    lx = s0in.tile([in_ch + 1, 128], f32, tag="lx")
                    nc.sync.dma_start(lx[:, :m], x_aug[:, base : base + m])
                    # l-branch: features + att-weighted row sums
                    psl = eps.tile([128, D], f32, space="PSUM", tag="out")
                    nc.tensor.matmul(
                        psl[:m, :], lhsT=lx[:, :m], rhs=w1l_sb[:], start=True, stop=True
                    )
                    sb = s0out.tile([128, W_TAB], f32, tag="sb")
                    nc.scalar.copy(sb[:m, :D], psl[:m, :])
                    tmp = s0out.tile([128, D], f32, tag="tmp")
                    nc.vector.tensor_mul(tmp[:m, :], sb[:m, :D], att1_sb[:m, :])
                    nc.vector.reduce_sum(
                        out=sb[:m, D:W_TAB],
                        in_=tmp[:m, :].rearrange("p (h c) -> p h c", h=HEADS),
                        axis=mybir.AxisListType.X,
                    )
                    nc.sync.dma_start(xl1_loc[base : base + m, :], sb[:m, :])
                    # r-branch: features only
                    psr = eps.tile([128, D], f32, space="PSUM", tag="out")
                    nc.tensor.matmul(
                        psr[:m, :], lhsT=lx[:, :m], rhs=w1r_sb[:], start=True, stop=True
                    )
                    sbr = s0out.tile([128, D], f32, tag="sbr")
                    nc.scalar.copy(sbr[:m, :], psr[:m, :])
                    nc.sync.dma_start(xr1_loc[base : base + m, :], sbr[:m, :])

            nc.gpsimd.collective_compute(
                "AllGather",
                mybir.AluOpType.bypass,
                replica_groups=[list(range(n_cores))],
                ins=[xl1_loc.opt()],
                outs=[xl1_full.opt()],
            )

            # ================= edge phase (shared pools, both layers)
            def edge_phase(xl_full_ap, xr_loc_ap, att_sb, bias_sb, hT_ap, segj):
                with seg_loop(segj):
                    for b in range(blocks):
                        tb = tiles[b]
                        off = offs[b]
                        cbase = b * BLOCK
                        cols = min(BLOCK, per - cbase)
                        src_sb = eidx.tile([128, tb], i32, tag="src")
                        nc.sync.dma_start(src_sb[:], esrc[:, off : off + tb])
                        dst_sb = eidx.tile([128, tb], i32, tag="dst")
                        nc.sync.dma_start(dst_sb[:], edst[:, off : off + tb])
                        lv_sb = eidx.tile([128, tb], f32, tag="lv")
                        nc.sync.dma_start(lv_sb[:], logvalid[:, off : off + tb])
                        seg_f = eidx.tile([128, tb], f32, tag="seg")
                        nc.vector.tensor_copy(seg_f[:], dst_sb[:])
                        nc.vector.tensor_scalar_add(seg_f[:], seg_f[:], float(-cbase))

                        xr_blk = exr.tile([128, D], f32, tag="xrb")
                        nc.sync.dma_start(xr_blk[:], xr_loc_ap[cbase : cbase + 128, :])

                        den_ps = eps.tile([128, 4], f32, space="PSUM", tag="den")
                        out_ps = eps.tile([128, D], f32, space="PSUM", tag="out")

                        for t in range(tb):
                            xg = eg.tile([128, W_TAB], f32, tag="xg")
                            nc.gpsimd.indirect_dma_start(
                                out=xg[:],
                                out_offset=None,
                                in_=xl_full_ap,
                                in_offset=bass.IndirectOffsetOnAxis(
                                    ap=src_sb[:, t : t + 1], axis=0
                                ),
                            )
                            oh = eoh.tile([128, 128], f32, tag="oh")
                            nc.vector.tensor_tensor(
                                out=oh[:],
                                in0=seg_f[:, t : t + 1].to_broadcast([128, 128]),
                                in1=iota_f[:],
                                op=mybir.AluOpType.is_equal,
                            )
                            ohT_ps = etps.tile([128, 128], f32, space="PSUM", tag="tp")
                            nc.tensor.transpose(ohT_ps[:], oh[:], identity[:])
                            ohT = eoh.tile([128, 128], f32, tag="ohT")
                            nc.vector.tensor_copy(ohT[:], ohT_ps[:])
                            xr_e = ebr.tile([128, D], f32, space="PSUM", tag="xre")
                            nc.tensor.matmul(
                                xr_e[:], lhsT=ohT[:], rhs=xr_blk[:], start=True, stop=True
                            )
                            z = ew.tile([128, D], f32, tag="z")
                            nc.vector.tensor_add(z[:], xr_e[:], xg[:, :D])
                            ab = ew.tile([128, D], f32, tag="ab")
                            nc.scalar.activation(
                                ab[:], z[:], mybir.ActivationFunctionType.Abs
                            )
                            nc.vector.tensor_mul(ab[:], ab[:], att_sb[:])
                            red = esm.tile([128, 4], f32, tag="red")
                            nc.vector.reduce_sum(
                                out=red[:],
                                in_=ab[:].rearrange("p (h c) -> p h c", h=HEADS),
                                axis=mybir.AxisListType.X,
                            )
                            pre = esm.tile([128, 4], f32, tag="pre")
                            nc.vector.tensor_scalar_mul(pre[:], xg[:, D:W_TAB], 1.5)
                            nc.vector.tensor_add(pre[:], pre[:], red[:])
                            exps = esm.tile([128, 4], f32, tag="exps")
                            nc.scalar.activation(
                                exps[:],
                                pre[:],
                                mybir.ActivationFunctionType.Exp,
                                bias=lv_sb[:, t : t + 1],
                                scale=0.4,
                            )
                            nc.tensor.matmul(
                                den_ps[:],
                                lhsT=oh[:],
                                rhs=exps[:],
                                start=(t == 0),
                                stop=(t == tb - 1),
                            )
                            w = ew.tile([128, D], f32, tag="w")
                            nc.vector.tensor_tensor(
                                out=w[:].rearrange("p (h c) -> p h c", h=HEADS),
                                in0=xg[:, :D].rearrange("p (h c) -> p h c", h=HEADS),
                                in1=exps[:, :, None].to_broadcast([128, HEADS, HID]),
                                op=mybir.AluOpType.mult,
                            )
                            nc.tensor.matmul(
                                out_ps[:],
                                lhsT=oh[:],
                                rhs=w[:],
                                start=(t == 0),
                                stop=(t == tb - 1),
                            )

                        # ---- block epilogue
                        den_sb = esm.tile([128, 4], f32, tag="den_sb")
                        nc.vector.tensor_copy(den_sb[:], den_ps[:])
                        recip = esm.tile([128, 4], f32, tag="recip")
                        nc.vector.reciprocal(recip[:], den_sb[:])
                        h = etail.tile([128, D], f32, tag="h")
                        nc.vector.tensor_tensor(
                            out=h[:].rearrange("p (h c) -> p h c", h=HEADS),
                            in0=out_ps[:].rearrange("p (h c) -> p h c", h=HEADS),
                            in1=recip[:, :, None].to_broadcast([128, HEADS, HID]),
                            op=mybir.AluOpType.mult,
                        )
                        nc.vector.tensor_add(h[:], h[:], bias_sb[:])
                        # ELU: relu(x) + exp(min(x,0)) - 1
                        neg = etail.tile([128, D], f32, tag="neg")
                        nc.vector.tensor_scalar_min(neg[:], h[:], 0.0)
                        expn = etail.tile([128, D], f32, tag="expn")
                        nc.scalar.activation(
                            expn[:], neg[:], mybir.ActivationFunctionType.Exp
                        )
                        pos = etail.tile([128, D], f32, tag="pos")
                        nc.scalar.activation(
                            pos[:], h[:], mybir.ActivationFunctionType.Relu
                        )
                        hf = etail.tile([128, D], f32, tag="hf")
                        nc.vector.tensor_add(hf[:], pos[:], expn[:])
                        nc.scalar.add(hf[:], hf[:], -1.0)
                        for q in range(4):
                            tp = etps.tile([128, 128], f32, space="PSUM", tag="tp")
                            nc.tensor.transpose(
                                tp[:], hf[:, q * 128 : (q + 1) * 128], identity[:]
                            )
                            tsb = etail.tile([128, 128], f32, tag="tsb")
                            nc.vector.tensor_copy(tsb[:], tp[:])
                            nc.sync.dma_start(
                                hT_ap[q * 128 : (q + 1) * 128, cbase : cbase + cols],
                                tsb[:, :cols],
                            )

            edge_phase(xl1_full.opt(), xr1_loc.opt(), att1_sb, bias1_sb, h1T.opt(), 1)

            # ================= stage 2: layer-2 dense transforms from h1T
            with (
                tc.tile_pool(name="s2_in", bufs=4) as s2in,
                tc.tile_pool(name="s2_out", bufs=3) as s2out,
                seg_loop(2),
            ):
                for base, m in node_tiles:
                    hts = []
                    for k in range(4):
                        ht = s2in.tile([128, 128], f32, tag=f"ht{k}")
                        nc.sync.dma_start(
                            ht[:, :m], h1T[k * 128 : (k + 1) * 128, base : base + m]
                        )
                        hts.append(ht)
                    # l-branch
                    psl = eps.tile([128, D], f32, space="PSUM", tag="out")
                    for k in range(4):
                        nc.tensor.matmul(
                            psl[:m, :], lhsT=hts[k][:, :m], rhs=w2l_ch[k][:],
                            start=(k == 0), stop=False,
                        )
                    nc.tensor.matmul(
                        psl[:m, :], lhsT=ones_sb[:, :m], rhs=b2l_sb[:],
                        start=False, stop=True,
                    )
                    sb = s2out.tile([128, W_TAB], f32, tag="sb")
                    nc.scalar.copy(sb[:m, :D], psl[:m, :])
                    tmp = s2out.tile([128, D], f32, tag="tmp")
                    nc.vector.tensor_mul(tmp[:m, :], sb[:m, :D], att2_sb[:m, :])
                    nc.vector.reduce_sum(
                        out=sb[:m, D:W_TAB],
                        in_=tmp[:m, :].rearrange("p (h c) -> p h c", h=HEADS),
                        axis=mybir.AxisListType.X,
                    )
                    nc.sync.dma_start(xl2_loc[base : base + m, :], sb[:m, :])
                    # r-branch
                    psr = eps.tile([128, D], f32, space="PSUM", tag="out")
                    for k in range(4):
                        nc.tensor.matmul(
                            psr[:m, :], lhsT=hts[k][:, :m], rhs=w2r_ch[k][:],
                            start=(k == 0), stop=False,
                        )
                    nc.tensor.matmul(
                        psr[:m, :], lhsT=ones_sb[:, :m], rhs=b2r_sb[:],
                        start=False, stop=True,
                    )
                    sbr = s2out.tile([128, D], f32, tag="sbr")
                    nc.scalar.copy(sbr[:m, :], psr[:m, :])
                    nc.sync.dma_start(xr2_loc[base : base + m, :], sbr[:m, :])

            nc.gpsimd.collective_compute(
                "AllGather",
                mybir.AluOpType.bypass,
                replica_groups=[list(range(n_cores))],
                ins=[xl2_loc.opt()],
                outs=[xl2_full.opt()],
            )

            edge_phase(xl2_full.opt(), xr2_loc.opt(), att2_sb, bias2_sb, h2T.opt(), 3)

            # ================= classifier
            with (
                tc.tile_pool(name="c_in", bufs=4) as cin,
                tc.tile_pool(name="c_out", bufs=3) as cout,
                seg_loop(4),
            ):
                for base, m in node_tiles:
                    hts = []
                    for k in range(4):
                        ht = cin.tile([128, 128], f32, tag=f"cht{k}")
                        nc.sync.dma_start(
                            ht[:, :m], h2T[k * 128 : (k + 1) * 128, base : base + m]
                        )
                        hts.append(ht)
                    ps = eps.tile([128, OUT_CH], f32, space="PSUM", tag="out")
                    for k in range(4):
                        nc.tensor.matmul(
                            ps[:m, :], lhsT=hts[k][:, :m], rhs=wcls_ch[k][:],
                            start=(k == 0), stop=False,
                        )
                    nc.tensor.matmul(
                        ps[:m, :], lhsT=ones_sb[:, :m], rhs=bcls_sb[:],
                        start=False, stop=True,
                    )
                    sb = cout.tile([128, OUT_CH], f32, tag="sb")
                    nc.scalar.copy(sb[:m, :], ps[:m, :])
                    nc.sync.dma_start(out[base : base + m, :], sb[:m, :])

    nc.compile()
    return nc


# ---------------------------------------------------------------- entry

_CACHE = {}


def kernel(**inputs):
    from concourse.bass_utils import run_bass_kernel_spmd

    in_maps, meta = preprocess(**inputs)
    key = (meta["tt"], tuple(meta["tiles"]))
    if key not in _CACHE:
        _CACHE[key] = build_program(meta)
    nc = _CACHE[key]
    res = run_bass_kernel_spmd(nc, in_maps, list(range(meta["n_cores"])))
    outs = [res.results[c]["out"] for c in range(meta["n_cores"])]
    return np.concatenate(outs, axis=0)


# ---------------------------------------------------------------- numpy model
# (host-side mirror of the device math, for validation in test.py)


def numpy_model(x, edge_index, w1_l, b1_l, w1_r, b1_r, att1, bias1,
                w2_l, b2_l, w2_r, b2_r, att2, bias2, w_cls, b_cls):
    x = np.asarray(x, np.float32)
    n = x.shape[0]
    ei = np.asarray(edge_index).astype(np.int64)
    loops = np.arange(n, dtype=np.int64)
    src = np.concatenate([ei[0], loops])
    dst = np.concatenate([ei[1], loops])

    def layer(h, wl, bl, wr, br, att, bias):
        att_flat = np.asarray(att, np.float32).reshape(-1)
        xl = (h @ np.asarray(wl, np.float32) + np.asarray(bl, np.float32)).astype(np.float32)
        xr = (h @ np.asarray(wr, np.float32) + np.asarray(br, np.float32)).astype(np.float32)
        adl = (xl * att_flat).reshape(n, HEADS, HID).sum(axis=2)
        z = xl[src] + xr[dst]
        abssum = (np.abs(z) * att_flat).reshape(-1, HEADS, HID).sum(axis=2)
        # adr[dst] is constant per softmax segment -> dropped (shift invariance)
        score = 0.4 * (abssum + 1.5 * adl[src])
        ex = np.exp(score)
        den = np.zeros((n, HEADS), np.float32)
        np.add.at(den, dst, ex)
        unnorm = np.zeros((n, HEADS, HID), np.float32)
        np.add.at(unnorm, dst, xl[src].reshape(-1, HEADS, HID) * ex[:, :, None])
        out = unnorm / den[:, :, None]
        out = out.reshape(n, D) + np.asarray(bias, np.float32)
        return np.where(out > 0, out, np.exp(np.minimum(out, 0)) - 1).astype(np.float32)

    h = layer(x, w1_l, b1_l, w1_r, b1_r, att1, bias1)
    h = layer(h, w2_l, b2_l, w2_r, b2_r, att2, bias2)
    return (h @ np.asarray(w_cls, np.float32) + np.asarray(b_cls, np.float32)).astype(
        np.float32
    )


# revision 13
# speedup vs baseline: 1.1251x; 1.1251x over previous
"""GATv2 (2-layer, 4 heads) + linear classifier on Trainium2, 8-core SPMD.

Sharding: nodes are partitioned contiguously across 8 cores (2500 nodes/core).
Edges are routed to the core that owns their destination node, so the
segment-softmax and scatter-add stay core-local.  The only cross-core
exchange is one AllGather per GAT layer of the source-side linear transform
table, which every core then gathers rows from by edge source id (the "halo
exchange" for cut edges).

Math decomposition (per layer, per head h with C=128 channels), using
leaky_relu(z) = 0.6 z + 0.4 |z| for slope 0.2:
  score_e = att_h . leaky_relu(z) = 0.6 * att.z + 0.4 * att.|z|
with z = xl[src] + xr[dst].  The src part of the linear term, adl[src] =
(att * xl).rowsum per head, is stored as 4 extra table columns so one gather
brings both the 512 features and the sums.  The dst part (adr[dst]) is
constant within each softmax segment and cancels, so it is dropped.
Indirect gathers are row-rate-limited (~12ns/row regardless of width), so
the xr[dst] side is never gathered: each 128-dst block loads its xr rows
with one direct DMA and broadcasts them to edges on the tensor engine using
the transposed one-hot (edge -> segment) matrix.
Softmax max-subtraction is skipped (scores are in [-6, 6], exact in fp32).
Per block, one-hot matmuls accumulate the exp-weight segment sum ("denom")
and the exp-weighted feature segment sum into PSUM; the block epilogue
divides by denom, adds the output bias, applies ELU, and stores the block
transposed (feature-major) for the next layer's matmuls.
"""

import numpy as np

N_NODES = 20000
IN_CH = 55
HID = 128
HEADS = 4
D = HID * HEADS  # 512
OUT_CH = 49
NCORES = 8
PER_CORE = N_NODES // NCORES  # 2500
BLOCK = 128
NEG_SLOPE = 0.2
W_TAB = D + HEADS  # 516: features + per-head att-weighted row sums


# ---------------------------------------------------------------- host prep


def _plan_edges(src, dst, n_nodes, per_core, n_cores):
    """Route edges to (core, block) by dst; pad each block to a multiple of
    128 edge slots, uniformly across cores (SPMD program must be identical).
    """
    blocks_per_core = (per_core + BLOCK - 1) // BLOCK
    core = dst // per_core
    dst_local = dst - core * per_core
    blk = dst_local // BLOCK

    counts = np.zeros((n_cores, blocks_per_core), dtype=np.int64)
    np.add.at(counts, (core, blk), 1)
    tiles = np.maximum(1, -(-counts.max(axis=0) // 128))
    offs = np.concatenate([[0], np.cumsum(tiles)])[:-1]
    tt = int(tiles.sum())

    esrc = np.zeros((n_cores, 128, tt), dtype=np.int32)
    edst = np.zeros((n_cores, 128, tt), dtype=np.int32)
    lv = np.full((n_cores, 128, tt), -1e30, dtype=np.float32)

    order = np.lexsort((blk, core))
    src_s, dstl_s = src[order], dst_local[order]
    key = core[order] * blocks_per_core + blk[order]
    bounds = np.searchsorted(key, np.arange(n_cores * blocks_per_core + 1))
    for c in range(n_cores):
        for b in range(blocks_per_core):
            k = c * blocks_per_core + b
            lo, hi = bounds[k], bounds[k + 1]
            cnt = hi - lo
            nslots = int(tiles[b]) * 128
            s = np.zeros(nslots, dtype=np.int32)
            d_ = np.zeros(nslots, dtype=np.int32)
            v = np.full(nslots, -1e30, dtype=np.float32)
            s[:cnt] = src_s[lo:hi]
            d_[:cnt] = dstl_s[lo:hi]
            d_[cnt:] = b * BLOCK  # dummy slots: stay inside this block
            v[:cnt] = 0.0
            o = int(offs[b])
            t = int(tiles[b])
            esrc[c][:, o : o + t] = s.reshape(t, 128).T
            edst[c][:, o : o + t] = d_.reshape(t, 128).T
            lv[c][:, o : o + t] = v.reshape(t, 128).T
    return dict(
        tiles=[int(t) for t in tiles],
        offs=[int(o) for o in offs],
        tt=tt,
        blocks=blocks_per_core,
        esrc=esrc,
        edst=edst,
        logvalid=lv,
    )


def _bf16(a):
    import ml_dtypes
    return np.asarray(a, np.float32).astype(ml_dtypes.bfloat16)


def preprocess(x, edge_index, w1_l, b1_l, w1_r, b1_r, att1, bias1,
               w2_l, b2_l, w2_r, b2_r, att2, bias2, w_cls, b_cls,
               n_cores=NCORES):
    x = np.asarray(x, np.float32)
    n = x.shape[0]
    per = n // n_cores
    ei = np.asarray(edge_index).astype(np.int64)
    loops = np.arange(n, dtype=np.int64)
    src = np.concatenate([ei[0], loops])
    dst = np.concatenate([ei[1], loops])

    att1_flat = np.asarray(att1, np.float32).reshape(-1)
    att2_flat = np.asarray(att2, np.float32).reshape(-1)

    plan = _plan_edges(src, dst, n, per, n_cores)

    in_ch = x.shape[1]
    aug = lambda w, b: np.concatenate(
        [np.asarray(w, np.float32), np.asarray(b, np.float32)[None, :]], axis=0
    )
    rep = lambda v: np.broadcast_to(
        np.asarray(v, np.float32)[None, :], (128, v.shape[0])
    ).copy()

    shared = {
        "w1l_aug": aug(w1_l, b1_l),
        "w1r_aug": aug(w1_r, b1_r),
        "w2l": np.asarray(w2_l, np.float32),
        "w2r": np.asarray(w2_r, np.float32),
        "b2l_row": np.asarray(b2_l, np.float32)[None, :],
        "b2r_row": np.asarray(b2_r, np.float32)[None, :],
        "wcls": np.pad(np.asarray(w_cls, np.float32), ((0, 0), (0, 64 - OUT_CH))),
        "bcls_row": np.pad(np.asarray(b_cls, np.float32), (0, 64 - OUT_CH))[None, :],
        "att1_rep": rep(att1_flat),
        "att2_rep": rep(att2_flat),
        "bias1_rep": rep(np.asarray(bias1, np.float32)),
        "bias2_rep": rep(np.asarray(bias2, np.float32)),
    }
    in_maps = []
    for c in range(n_cores):
        xa = np.concatenate(
            [x[c * per : (c + 1) * per].T, np.ones((1, per), np.float32)], axis=0
        )
        m = dict(shared)
        m["x_aug"] = np.ascontiguousarray(xa)
        m["esrc"] = plan["esrc"][c]
        m["edst"] = plan["edst"][c]
        m["logvalid"] = plan["logvalid"][c]
        in_maps.append(m)
    meta = dict(
        n=n, per=per, in_ch=in_ch, tiles=plan["tiles"], offs=plan["offs"],
        tt=plan["tt"], blocks=plan["blocks"], n_cores=n_cores,
    )
    return in_maps, meta


# ---------------------------------------------------------------- device


def build_program(meta, bench=False):
    import contextlib
    import concourse.bass as bass
    import concourse.tile as tile
    import concourse.mybir as mybir
    from concourse import bacc
    from concourse.masks import make_identity

    f32 = mybir.dt.float32
    bf16 = mybir.dt.bfloat16
    f32r = mybir.dt.float32r
    i32 = mybir.dt.int32

    n = meta["n"]
    per = meta["per"]
    in_ch = meta["in_ch"]
    tiles = meta["tiles"]
    offs = meta["offs"]
    tt = meta["tt"]
    blocks = meta["blocks"]
    n_cores = meta["n_cores"]
    perp = blocks * BLOCK  # xr tables padded to full blocks

    nt_full, nt_rem = divmod(per, 128)
    node_tiles = [(i * 128, 128) for i in range(nt_full)]
    if nt_rem:
        node_tiles.append((nt_full * 128, nt_rem))

    nc = bacc.Bacc("TRN2", target_bir_lowering=False, debug=False, num_devices=n_cores)

    # register a -1.0 const AP so scalar.add(x, -1.0) lowers on the ACT engine
    _cm1 = nc.alloc_sbuf_tensor("const-float32-neg1", [128, 1], f32)
    nc.gpsimd.memset(_cm1.ap(), -1.0)
    nc.const_aps.aps[(f32, -1.0)] = _cm1.ap()

    def din(name, shape, dt=f32):
        return nc.dram_tensor(name, shape, dt, kind="ExternalInput").ap()

    x_aug = din("x_aug", [in_ch + 1, per], f32r)
    w1l_aug = din("w1l_aug", [in_ch + 1, D], f32r)
    w1r_aug = din("w1r_aug", [in_ch + 1, D], f32r)
    w2l = din("w2l", [D, D], f32r)
    w2r = din("w2r", [D, D], f32r)
    b2l_row = din("b2l_row", [1, D], f32r)
    b2r_row = din("b2r_row", [1, D], f32r)
    wcls = din("wcls", [D, 64], f32r)
    bcls_row = din("bcls_row", [1, 64], f32r)
    att1_rep = din("att1_rep", [128, D])
    att2_rep = din("att2_rep", [128, D])
    bias1_rep = din("bias1_rep", [128, D])
    bias2_rep = din("bias2_rep", [128, D])
    esrc = din("esrc", [128, tt], i32)
    edst = din("edst", [128, tt], i32)
    logvalid = din("logvalid", [128, tt])
    kreps = din("kreps", [1, 8], i32) if bench else None

    out = nc.dram_tensor("out", [per, OUT_CH], f32, kind="ExternalOutput").ap()

    with tile.TileContext(nc) as tc:
        with (
            tc.tile_pool(name="dram", bufs=1, space="DRAM") as dram,
            tc.tile_pool(name="consts", bufs=1) as consts,
            # ---- edge-phase pools, shared by both layers
            tc.tile_pool(name="eidx", bufs=2) as eidx,
            tc.tile_pool(name="eg", bufs=8) as eg,
            tc.tile_pool(name="esm", bufs=8) as esm,
            tc.tile_pool(name="eoh", bufs=4) as eoh,
            tc.tile_pool(name="ew", bufs=4) as ew,
            tc.tile_pool(name="exr", bufs=2) as exr,
            tc.tile_pool(name="eps", bufs=2, space="PSUM") as eps,
            tc.tile_pool(name="ebr", bufs=2, space="PSUM") as ebr,
            tc.tile_pool(name="etps", bufs=2, space="PSUM") as etps,
            tc.tile_pool(name="etail", bufs=2) as etail,
        ):
            # ---------- persistent DRAM intermediates
            xl1_loc = dram.tile([per, W_TAB], f32)
            xr1_loc = dram.tile([perp, D], f32r)
            xl1_full = dram.tile([n, W_TAB], f32, addr_space="Shared")
            h1T = dram.tile([D, perp], f32r)
            xl2_loc = dram.tile([per, W_TAB], f32)
            xr2_loc = dram.tile([perp, D], f32r)
            xl2_full = dram.tile([n, W_TAB], f32, addr_space="Shared")
            h2T = dram.tile([D, perp], f32r)

            # ---------- constants in SBUF
            identity = consts.tile([128, 128], f32)
            make_identity(nc, identity[:])
            id_r_t = consts.tile([128, 128], f32r)
            nc.vector.tensor_copy(id_r_t[:], identity[:])
            id_r = id_r_t[:]
            iota_f = consts.tile([128, 128], f32)
            iota_i = consts.tile([128, 128], i32)
            nc.gpsimd.iota(iota_i[:], pattern=[[1, 128]], base=0, channel_multiplier=0)
            nc.vector.tensor_copy(iota_f[:], iota_i[:])
            ones_f = consts.tile([1, 128], f32)
            nc.vector.memset(ones_f[:], 1.0)
            ones_sb = consts.tile([1, 128], f32r)
            nc.vector.tensor_copy(ones_sb[:], ones_f[:])

            if bench:
                kt = consts.tile([1, 8], i32)
                nc.sync.dma_start(kt[:], kreps[:])
                kregs = [nc.values_load(kt[0:1, j : j + 1]) for j in range(5)]

            def seg_loop(j):
                if bench:
                    return tc.For_i(0, kregs[j], 1)
                return contextlib.nullcontext()

            w1l_sb = consts.tile([in_ch + 1, D], f32r)
            nc.sync.dma_start(w1l_sb[:], w1l_aug[:])
            w1r_sb = consts.tile([in_ch + 1, D], f32r)
            nc.sync.dma_start(w1r_sb[:], w1r_aug[:])
            att1_sb = consts.tile([128, D], f32)
            nc.sync.dma_start(att1_sb[:], att1_rep[:])
            att2_sb = consts.tile([128, D], f32)
            nc.sync.dma_start(att2_sb[:], att2_rep[:])
            bias1_sb = consts.tile([128, D], f32)
            nc.sync.dma_start(bias1_sb[:], bias1_rep[:])
            bias2_sb = consts.tile([128, D], f32)
            nc.sync.dma_start(bias2_sb[:], bias2_rep[:])

            w2l_ch = []
            w2r_ch = []
            wcls_ch = []
            for k in range(4):
                t1 = consts.tile([128, D], f32r, name=f"w2l_{k}")
                nc.sync.dma_start(t1[:], w2l[k * 128 : (k + 1) * 128, :])
                w2l_ch.append(t1)
                t2 = consts.tile([128, D], f32r, name=f"w2r_{k}")
                nc.sync.dma_start(t2[:], w2r[k * 128 : (k + 1) * 128, :])
                w2r_ch.append(t2)
                t3 = consts.tile([128, 64], f32r, name=f"wcls_{k}")
                nc.sync.dma_start(t3[:], wcls[k * 128 : (k + 1) * 128, :])
                wcls_ch.append(t3)
            b2l_sb = consts.tile([1, D], f32r)
            nc.sync.dma_start(b2l_sb[:], b2l_row[:])
            b2r_sb = consts.tile([1, D], f32r)
            nc.sync.dma_start(b2r_sb[:], b2r_row[:])
            bcls_sb = consts.tile([1, 64], f32r)
            nc.sync.dma_start(bcls_sb[:], bcls_row[:])

            # zero-fill the xr tables' padded tail rows once
            if perp > per:
                zpad_f = consts.tile([128, D], f32)
                nc.vector.memset(zpad_f[:], 0.0)
                zpad = consts.tile([128, D], f32r)
                nc.vector.tensor_copy(zpad[:], zpad_f[:])
                nc.sync.dma_start(xr1_loc[per:perp, :], zpad[: perp - per, :])
                nc.sync.dma_start(xr2_loc[per:perp, :], zpad[: perp - per, :])

            # ================= stage 0: layer-1 dense transforms (local rows)
            with (
                tc.tile_pool(name="s0_in", bufs=3) as s0in,
                tc.tile_pool(name="s0_out", bufs=3) as s0out,
                seg_loop(0),
            ):
                for base, m in node_tiles:
                    lx = s0in.tile([in_ch + 1, 128], f32r, tag="lx")
                    nc.sync.dma_start(lx[:, :m], x_aug[:, base : base + m])
                    # l-branch: features + att-weighted row sums
                    psl = eps.tile([128, D], f32, space="PSUM", tag="out")
                    nc.tensor.matmul(
                        psl[:m, :], lhsT=lx[:, :m],
                        rhs=w1l_sb[:], start=True, stop=True,
                    )
                    sb = s0out.tile([128, W_TAB], f32, tag="sb")
                    nc.scalar.copy(sb[:m, :D], psl[:m, :])
                    tmp = s0out.tile([128, D], f32, tag="tmp")
                    nc.vector.tensor_mul(tmp[:m, :], sb[:m, :D], att1_sb[:m, :])
                    nc.vector.reduce_sum(
                        out=sb[:m, D:W_TAB],
                        in_=tmp[:m, :].rearrange("p (h c) -> p h c", h=HEADS),
                        axis=mybir.AxisListType.X,
                    )
                    nc.sync.dma_start(xl1_loc[base : base + m, :], sb[:m, :])
                    # r-branch: features only
                    psr = eps.tile([128, D], f32, space="PSUM", tag="out")
                    nc.tensor.matmul(
                        psr[:m, :], lhsT=lx[:, :m],
                        rhs=w1r_sb[:], start=True, stop=True,
                    )
                    sbr = s0out.tile([128, D], f32r, tag="sbr")
                    nc.scalar.copy(sbr[:m, :], psr[:m, :])
                    nc.sync.dma_start(xr1_loc[base : base + m, :], sbr[:m, :])

            nc.gpsimd.collective_compute(
                "AllGather",
                mybir.AluOpType.bypass,
                replica_groups=[list(range(n_cores))],
                ins=[xl1_loc.opt()],
                outs=[xl1_full.opt()],
            )

            # ================= edge phase (shared pools, both layers)
            def edge_phase(xl_full_ap, xr_loc_ap, att_sb, bias_sb, hT_ap, segj):
                with seg_loop(segj):
                    for b in range(blocks):
                        tb = tiles[b]
                        off = offs[b]
                        cbase = b * BLOCK
                        cols = min(BLOCK, per - cbase)
                        src_sb = eidx.tile([128, tb], i32, tag="src")
                        nc.sync.dma_start(src_sb[:], esrc[:, off : off + tb])
                        dst_sb = eidx.tile([128, tb], i32, tag="dst")
                        nc.sync.dma_start(dst_sb[:], edst[:, off : off + tb])
                        lv_sb = eidx.tile([128, tb], f32, tag="lv")
                        nc.sync.dma_start(lv_sb[:], logvalid[:, off : off + tb])
                        seg_f = eidx.tile([128, tb], f32, tag="seg")
                        nc.vector.tensor_copy(seg_f[:], dst_sb[:])
                        nc.vector.tensor_scalar_add(seg_f[:], seg_f[:], float(-cbase))

                        xr_blk = exr.tile([128, D], f32r, tag="xrb")
                        nc.sync.dma_start(xr_blk[:], xr_loc_ap[cbase : cbase + 128, :])

                        den_ps = eps.tile([128, 4], f32, space="PSUM", tag="den")
                        out_ps = eps.tile([128, D], f32, space="PSUM", tag="out")

                        for t in range(tb):
                            xg = eg.tile([128, W_TAB], f32, tag="xg")
                            nc.gpsimd.indirect_dma_start(
                                out=xg[:],
                                out_offset=None,
                                in_=xl_full_ap,
                                in_offset=bass.IndirectOffsetOnAxis(
                                    ap=src_sb[:, t : t + 1], axis=0
                                ),
                            )
                            oh = eoh.tile([128, 128], f32r, tag="oh")
                            nc.vector.tensor_tensor(
                                out=oh[:],
                                in0=seg_f[:, t : t + 1].to_broadcast([128, 128]),
                                in1=iota_f[:],
                                op=mybir.AluOpType.is_equal,
                            )
                            ohT_ps = etps.tile([128, 128], f32r, space="PSUM", tag="tp")
                            nc.tensor.transpose(ohT_ps[:], oh[:], id_r)
                            ohT = eoh.tile([128, 128], f32r, tag="ohT")
                            nc.vector.tensor_copy(ohT[:], ohT_ps[:])
                            xr_e = ebr.tile([128, D], f32, space="PSUM", tag="xre")
                            nc.tensor.matmul(
                                xr_e[:], lhsT=ohT[:],
                                rhs=xr_blk[:], start=True, stop=True,
                            )
                            z = ew.tile([128, D], f32, tag="z")
                            nc.vector.tensor_add(z[:], xr_e[:], xg[:, :D])
                            ab = ew.tile([128, D], f32, tag="ab")
                            nc.scalar.activation(
                                ab[:], z[:], mybir.ActivationFunctionType.Abs
                            )
                            nc.vector.tensor_mul(ab[:], ab[:], att_sb[:])
                            red = esm.tile([128, 4], f32, tag="red")
                            nc.vector.reduce_sum(
                                out=red[:],
                                in_=ab[:].rearrange("p (h c) -> p h c", h=HEADS),
                                axis=mybir.AxisListType.X,
                            )
                            pre = esm.tile([128, 4], f32, tag="pre")
                            nc.vector.tensor_scalar_mul(pre[:], xg[:, D:W_TAB], 1.5)
                            nc.vector.tensor_add(pre[:], pre[:], red[:])
                            exps = esm.tile([128, 4], f32r, tag="exps")
                            nc.scalar.activation(
                                exps[:],
                                pre[:],
                                mybir.ActivationFunctionType.Exp,
                                bias=lv_sb[:, t : t + 1],
                                scale=0.4,
                            )
                            nc.tensor.matmul(
                                den_ps[:],
                                lhsT=oh[:],
                                rhs=exps[:],
                                start=(t == 0),
                                stop=(t == tb - 1),
                            )
                            w = ew.tile([128, D], f32r, tag="w")
                            nc.vector.tensor_tensor(
                                out=w[:].rearrange("p (h c) -> p h c", h=HEADS),
                                in0=xg[:, :D].rearrange("p (h c) -> p h c", h=HEADS),
                                in1=exps[:, :, None].to_broadcast([128, HEADS, HID]),
                                op=mybir.AluOpType.mult,
                            )
                            nc.tensor.matmul(
                                out_ps[:],
                                lhsT=oh[:],
                                rhs=w[:],
                                start=(t == 0),
                                stop=(t == tb - 1),
                            )

                        # ---- block epilogue
                        den_sb = esm.tile([128, 4], f32, tag="den_sb")
                        nc.vector.tensor_copy(den_sb[:], den_ps[:])
                        recip = esm.tile([128, 4], f32, tag="recip")
                        nc.vector.reciprocal(recip[:], den_sb[:])
                        h = etail.tile([128, D], f32, tag="h")
                        nc.vector.tensor_tensor(
                            out=h[:].rearrange("p (h c) -> p h c", h=HEADS),
                            in0=out_ps[:].rearrange("p (h c) -> p h c", h=HEADS),
                            in1=recip[:, :, None].to_broadcast([128, HEADS, HID]),
                            op=mybir.AluOpType.mult,
                        )
                        nc.vector.tensor_add(h[:], h[:], bias_sb[:])
                        # ELU: relu(x) + exp(min(x,0)) - 1
                        neg = etail.tile([128, D], f32, tag="neg")
                        nc.vector.tensor_scalar_min(neg[:], h[:], 0.0)
                        expn = etail.tile([128, D], f32, tag="expn")
                        nc.scalar.activation(
                            expn[:], neg[:], mybir.ActivationFunctionType.Exp
                        )
                        pos = etail.tile([128, D], f32, tag="pos")
                        nc.scalar.activation(
                            pos[:], h[:], mybir.ActivationFunctionType.Relu
                        )
                        hf = etail.tile([128, D], f32r, tag="hf")
                        nc.vector.tensor_add(hf[:], pos[:], expn[:])
                        nc.scalar.add(hf[:], hf[:], -1.0)
                        for q in range(4):
                            tp = etps.tile([128, 128], f32r, space="PSUM", tag="tp")
                            nc.tensor.transpose(
                                tp[:], hf[:, q * 128 : (q + 1) * 128], id_r
                            )
                            tsb = etail.tile([128, 128], f32r, tag="tsb")
                            nc.vector.tensor_copy(tsb[:], tp[:])
                            nc.sync.dma_start(
                                hT_ap[q * 128 : (q + 1) * 128, cbase : cbase + cols],
                                tsb[:, :cols],
                            )

            edge_phase(xl1_full.opt(), xr1_loc.opt(), att1_sb, bias1_sb, h1T.opt(), 1)

            # ================= stage 2: layer-2 dense transforms from h1T
            with (
                tc.tile_pool(name="s2_in", bufs=4) as s2in,
                tc.tile_pool(name="s2_out", bufs=3) as s2out,
                seg_loop(2),
            ):
                for base, m in node_tiles:
                    hts = []
                    for k in range(4):
                        ht = s2in.tile([128, 128], f32r, tag=f"ht{k}")
                        nc.sync.dma_start(
                            ht[:, :m], h1T[k * 128 : (k + 1) * 128, base : base + m]
                        )
                        hts.append(ht)
                    # l-branch
                    psl = eps.tile([128, D], f32, space="PSUM", tag="out")
                    for k in range(4):
                        nc.tensor.matmul(
                            psl[:m, :], lhsT=hts[k][:, :m],
                            rhs=w2l_ch[k][:],
                            start=(k == 0), stop=False,
                        )
                    nc.tensor.matmul(
                        psl[:m, :], lhsT=ones_sb[:, :m],
                        rhs=b2l_sb[:], start=False, stop=True,
                    )
                    sb = s2out.tile([128, W_TAB], f32, tag="sb")
                    nc.scalar.copy(sb[:m, :D], psl[:m, :])
                    tmp = s2out.tile([128, D], f32, tag="tmp")
                    nc.vector.tensor_mul(tmp[:m, :], sb[:m, :D], att2_sb[:m, :])
                    nc.vector.reduce_sum(
                        out=sb[:m, D:W_TAB],
                        in_=tmp[:m, :].rearrange("p (h c) -> p h c", h=HEADS),
                        axis=mybir.AxisListType.X,
                    )
                    nc.sync.dma_start(xl2_loc[base : base + m, :], sb[:m, :])
                    # r-branch
                    psr = eps.tile([128, D], f32, space="PSUM", tag="out")
                    for k in range(4):
                        nc.tensor.matmul(
                            psr[:m, :], lhsT=hts[k][:, :m],
                            rhs=w2r_ch[k][:],
                            start=(k == 0), stop=False,
                        )
                    nc.tensor.matmul(
                        psr[:m, :], lhsT=ones_sb[:, :m],
                        rhs=b2r_sb[:], start=False, stop=True,
                    )
                    sbr = s2out.tile([128, D], f32r, tag="sbr")
                    nc.scalar.copy(sbr[:m, :], psr[:m, :])
                    nc.sync.dma_start(xr2_loc[base : base + m, :], sbr[:m, :])

            nc.gpsimd.collective_compute(
                "AllGather",
                mybir.AluOpType.bypass,
                replica_groups=[list(range(n_cores))],
                ins=[xl2_loc.opt()],
                outs=[xl2_full.opt()],
            )

            edge_phase(xl2_full.opt(), xr2_loc.opt(), att2_sb, bias2_sb, h2T.opt(), 3)

            # ================= classifier
            with (
                tc.tile_pool(name="c_in", bufs=4) as cin,
                tc.tile_pool(name="c_out", bufs=3) as cout,
                seg_loop(4),
            ):
                for base, m in node_tiles:
                    hts = []
                    for k in range(4):
                        ht = cin.tile([128, 128], f32r, tag=f"cht{k}")
                        nc.sync.dma_start(
                            ht[:, :m], h2T[k * 128 : (k + 1) * 128, base : base + m]
                        )
                        hts.append(ht)
                    ps = eps.tile([128, 64], f32, space="PSUM", tag="out")
                    for k in range(4):
                        nc.tensor.matmul(
                            ps[:m, :], lhsT=hts[k][:, :m],
                            rhs=wcls_ch[k][:],
                            start=(k == 0), stop=False,
                        )
                    nc.tensor.matmul(
                        ps[:m, :], lhsT=ones_sb[:, :m],
                        rhs=bcls_sb[:], start=False, stop=True,
                    )
                    sb = cout.tile([128, 64], f32, tag="sb")
                    nc.scalar.copy(sb[:m, :], ps[:m, :])
                    nc.sync.dma_start(out[base : base + m, :], sb[:m, :OUT_CH])

    nc.compile()
    return nc


# ---------------------------------------------------------------- entry

_CACHE = {}


def kernel(**inputs):
    from concourse.bass_utils import run_bass_kernel_spmd

    in_maps, meta = preprocess(**inputs)
    key = (meta["tt"], tuple(meta["tiles"]))
    if key not in _CACHE:
        _CACHE[key] = build_program(meta)
    nc = _CACHE[key]
    res = run_bass_kernel_spmd(nc, in_maps, list(range(meta["n_cores"])))
    outs = [res.results[c]["out"] for c in range(meta["n_cores"])]
    return np.concatenate(outs, axis=0)


# ---------------------------------------------------------------- numpy model
# (host-side mirror of the device math, for validation in test.py)


def numpy_model(x, edge_index, w1_l, b1_l, w1_r, b1_r, att1, bias1,
                w2_l, b2_l, w2_r, b2_r, att2, bias2, w_cls, b_cls):
    x = np.asarray(x, np.float32)
    n = x.shape[0]
    ei = np.asarray(edge_index).astype(np.int64)
    loops = np.arange(n, dtype=np.int64)
    src = np.concatenate([ei[0], loops])
    dst = np.concatenate([ei[1], loops])

    def layer(h, wl, bl, wr, br, att, bias):
        att_flat = np.asarray(att, np.float32).reshape(-1)
        xl = (h @ np.asarray(wl, np.float32) + np.asarray(bl, np.float32)).astype(np.float32)
        xr = (h @ np.asarray(wr, np.float32) + np.asarray(br, np.float32)).astype(np.float32)
        adl = (xl * att_flat).reshape(n, HEADS, HID).sum(axis=2)
        z = xl[src] + xr[dst]
        abssum = (np.abs(z) * att_flat).reshape(-1, HEADS, HID).sum(axis=2)
        # adr[dst] is constant per softmax segment -> dropped (shift invariance)
        score = 0.4 * (abssum + 1.5 * adl[src])
        ex = np.exp(score)
        den = np.zeros((n, HEADS), np.float32)
        np.add.at(den, dst, ex)
        unnorm = np.zeros((n, HEADS, HID), np.float32)
        np.add.at(unnorm, dst, xl[src].reshape(-1, HEADS, HID) * ex[:, :, None])
        out = unnorm / den[:, :, None]
        out = out.reshape(n, D) + np.asarray(bias, np.float32)
        return np.where(out > 0, out, np.exp(np.minimum(out, 0)) - 1).astype(np.float32)

    h = layer(x, w1_l, b1_l, w1_r, b1_r, att1, bias1)
    h = layer(h, w2_l, b2_l, w2_r, b2_r, att2, bias2)
    return (h @ np.asarray(w_cls, np.float32) + np.asarray(b_cls, np.float32)).astype(
        np.float32
    )


# revision 18
# speedup vs baseline: 1.2660x; 1.1252x over previous
"""GATv2 (2-layer, 4 heads) + linear classifier on Trainium2, 8-core SPMD.

Sharding: nodes are partitioned contiguously across 8 cores (2500 nodes/core).
Edges are routed to the core that owns their destination node, so the
segment-softmax and scatter-add stay core-local.  The only cross-core
exchange is one AllGather per GAT layer of the source-side linear transform
table, which every core then gathers rows from by edge source id (the "halo
exchange" for cut edges).

Math decomposition (per layer, per head h with C=128 channels), using
leaky_relu(z) = 0.6 z + 0.4 |z| for slope 0.2:
  score_e = att_h . leaky_relu(z) = 0.6 * att.z + 0.4 * att.|z|
with z = xl[src] + xr[dst].  The src part of the linear term, adl[src] =
(att * xl).rowsum per head, is stored as 4 extra table columns so one gather
brings both the 512 features and the sums.  The dst part (adr[dst]) is
constant within each softmax segment and cancels, so it is dropped.
Indirect gathers are row-rate-limited (~12ns/row regardless of width), so
the xr[dst] side is never gathered: each 128-dst block loads its xr rows
with one direct DMA and broadcasts them to edges on the tensor engine using
the transposed one-hot (edge -> segment) matrix.
Softmax max-subtraction is skipped (scores are in [-6, 6], exact in fp32).
Per block, one-hot matmuls accumulate the exp-weight segment sum ("denom")
and the exp-weighted feature segment sum into PSUM; the block epilogue
divides by denom, adds the output bias, applies ELU, and stores the block
transposed (feature-major) for the next layer's matmuls.
"""

import numpy as np

N_NODES = 20000
IN_CH = 55
HID = 128
HEADS = 4
D = HID * HEADS  # 512
OUT_CH = 49
NCORES = 8
PER_CORE = N_NODES // NCORES  # 2500
BLOCK = 128
NEG_SLOPE = 0.2
W_TAB = D + HEADS  # 516: features + per-head att-weighted row sums


# ---------------------------------------------------------------- host prep


def _plan_edges(src, dst, n_nodes, per_core, n_cores):
    """Route edges to (core, block) by dst; pad each block to a multiple of
    128 edge slots, uniformly across cores (SPMD program must be identical).
    """
    blocks_per_core = (per_core + BLOCK - 1) // BLOCK
    core = dst // per_core
    dst_local = dst - core * per_core
    blk = dst_local // BLOCK

    counts = np.zeros((n_cores, blocks_per_core), dtype=np.int64)
    np.add.at(counts, (core, blk), 1)
    tiles = np.maximum(1, -(-counts.max(axis=0) // 128))
    offs = np.concatenate([[0], np.cumsum(tiles)])[:-1]
    tt = int(tiles.sum())

    esrc = np.zeros((n_cores, 128, tt), dtype=np.int32)
    edst = np.zeros((n_cores, 128, tt), dtype=np.int32)
    lv = np.full((n_cores, 128, tt), -1e30, dtype=np.float32)

    order = np.lexsort((blk, core))
    src_s, dstl_s = src[order], dst_local[order]
    key = core[order] * blocks_per_core + blk[order]
    bounds = np.searchsorted(key, np.arange(n_cores * blocks_per_core + 1))
    for c in range(n_cores):
        for b in range(blocks_per_core):
            k = c * blocks_per_core + b
            lo, hi = bounds[k], bounds[k + 1]
            cnt = hi - lo
            nslots = int(tiles[b]) * 128
            s = np.zeros(nslots, dtype=np.int32)
            d_ = np.zeros(nslots, dtype=np.int32)
            v = np.full(nslots, -1e30, dtype=np.float32)
            s[:cnt] = src_s[lo:hi]
            d_[:cnt] = dstl_s[lo:hi]
            d_[cnt:] = b * BLOCK  # dummy slots: stay inside this block
            v[:cnt] = 0.0
            o = int(offs[b])
            t = int(tiles[b])
            esrc[c][:, o : o + t] = s.reshape(t, 128).T
            edst[c][:, o : o + t] = d_.reshape(t, 128).T
            lv[c][:, o : o + t] = v.reshape(t, 128).T
    return dict(
        tiles=[int(t) for t in tiles],
        offs=[int(o) for o in offs],
        tt=tt,
        blocks=blocks_per_core,
        esrc=esrc,
        edst=edst,
        logvalid=lv,
    )


def _bf16(a):
    import ml_dtypes
    return np.asarray(a, np.float32).astype(ml_dtypes.bfloat16)


def preprocess(x, edge_index, w1_l, b1_l, w1_r, b1_r, att1, bias1,
               w2_l, b2_l, w2_r, b2_r, att2, bias2, w_cls, b_cls,
               n_cores=NCORES):
    x = np.asarray(x, np.float32)
    n = x.shape[0]
    per = n // n_cores
    ei = np.asarray(edge_index).astype(np.int64)
    loops = np.arange(n, dtype=np.int64)
    src = np.concatenate([ei[0], loops])
    dst = np.concatenate([ei[1], loops])

    att1_flat = np.asarray(att1, np.float32).reshape(-1)
    att2_flat = np.asarray(att2, np.float32).reshape(-1)

    plan = _plan_edges(src, dst, n, per, n_cores)

    in_ch = x.shape[1]
    aug = lambda w, b: np.concatenate(
        [np.asarray(w, np.float32), np.asarray(b, np.float32)[None, :]], axis=0
    )
    rep = lambda v: np.broadcast_to(
        np.asarray(v, np.float32)[None, :], (128, v.shape[0])
    ).copy()

    shared = {
        "w1l_aug": aug(w1_l, b1_l),
        "w1r_aug": aug(w1_r, b1_r),
        "w2l": np.asarray(w2_l, np.float32),
        "w2r": np.asarray(w2_r, np.float32),
        "b2l_row": np.asarray(b2_l, np.float32)[None, :],
        "b2r_row": np.asarray(b2_r, np.float32)[None, :],
        "wcls": np.pad(np.asarray(w_cls, np.float32), ((0, 0), (0, 64 - OUT_CH))),
        "bcls_row": np.pad(np.asarray(b_cls, np.float32), (0, 64 - OUT_CH))[None, :],
        "att1_rep": rep(att1_flat),
        "att2_rep": rep(att2_flat),
        "att15_1_rep": rep(1.5 * att1_flat),
        "att15_2_rep": rep(1.5 * att2_flat),
        "bias1_rep": rep(np.asarray(bias1, np.float32)),
        "bias2_rep": rep(np.asarray(bias2, np.float32)),
    }
    in_maps = []
    for c in range(n_cores):
        xa = np.concatenate(
            [x[c * per : (c + 1) * per].T, np.ones((1, per), np.float32)], axis=0
        )
        m = dict(shared)
        m["x_aug"] = np.ascontiguousarray(xa)
        m["esrc"] = plan["esrc"][c]
        m["edst"] = plan["edst"][c]
        m["logvalid"] = plan["logvalid"][c]
        in_maps.append(m)
    meta = dict(
        n=n, per=per, in_ch=in_ch, tiles=plan["tiles"], offs=plan["offs"],
        tt=plan["tt"], blocks=plan["blocks"], n_cores=n_cores,
    )
    return in_maps, meta


# ---------------------------------------------------------------- device


def build_program(meta, bench=False):
    import contextlib
    import concourse.bass as bass
    import concourse.tile as tile
    import concourse.mybir as mybir
    from concourse import bacc
    from concourse.masks import make_identity

    f32 = mybir.dt.float32
    bf16 = mybir.dt.bfloat16
    f32r = mybir.dt.float32r
    i32 = mybir.dt.int32

    n = meta["n"]
    per = meta["per"]
    in_ch = meta["in_ch"]
    tiles = meta["tiles"]
    offs = meta["offs"]
    tt = meta["tt"]
    blocks = meta["blocks"]
    n_cores = meta["n_cores"]
    perp = blocks * BLOCK  # xr tables padded to full blocks

    nt_full, nt_rem = divmod(per, 128)
    node_tiles = [(i * 128, 128) for i in range(nt_full)]
    if nt_rem:
        node_tiles.append((nt_full * 128, nt_rem))

    nc = bacc.Bacc("TRN2", target_bir_lowering=False, debug=False, num_devices=n_cores)

    # register a -1.0 const AP so scalar.add(x, -1.0) lowers on the ACT engine
    _cm1 = nc.alloc_sbuf_tensor("const-float32-neg1", [128, 1], f32)
    nc.gpsimd.memset(_cm1.ap(), -1.0)
    nc.const_aps.aps[(f32, -1.0)] = _cm1.ap()

    def din(name, shape, dt=f32):
        return nc.dram_tensor(name, shape, dt, kind="ExternalInput").ap()

    x_aug = din("x_aug", [in_ch + 1, per], f32r)
    w1l_aug = din("w1l_aug", [in_ch + 1, D], f32r)
    w1r_aug = din("w1r_aug", [in_ch + 1, D], f32r)
    w2l = din("w2l", [D, D], f32r)
    w2r = din("w2r", [D, D], f32r)
    b2l_row = din("b2l_row", [1, D], f32r)
    b2r_row = din("b2r_row", [1, D], f32r)
    wcls = din("wcls", [D, 64], f32r)
    bcls_row = din("bcls_row", [1, 64], f32r)
    att1_rep = din("att1_rep", [128, D])
    att2_rep = din("att2_rep", [128, D])
    att15_1_rep = din("att15_1_rep", [128, D])
    att15_2_rep = din("att15_2_rep", [128, D])
    bias1_rep = din("bias1_rep", [128, D])
    bias2_rep = din("bias2_rep", [128, D])
    esrc = din("esrc", [128, tt], i32)
    edst = din("edst", [128, tt], i32)
    logvalid = din("logvalid", [128, tt])
    kreps = din("kreps", [1, 8], i32) if bench else None

    out = nc.dram_tensor("out", [per, OUT_CH], f32, kind="ExternalOutput").ap()
    dbg_h1T = (
        nc.dram_tensor("dbg_h1T", [D, blocks * BLOCK], f32, kind="ExternalOutput").ap()
        if bench
        else None
    )
    dbg_xl2 = (
        nc.dram_tensor("dbg_xl2", [per, W_TAB], f32, kind="ExternalOutput").ap()
        if bench
        else None
    )
    dbg_h2T = (
        nc.dram_tensor("dbg_h2T", [D, blocks * BLOCK], f32, kind="ExternalOutput").ap()
        if bench
        else None
    )

    with tile.TileContext(nc) as tc:
        with (
            tc.tile_pool(name="dram", bufs=1, space="DRAM") as dram,
            tc.tile_pool(name="consts", bufs=1) as consts,
            # ---- edge-phase pools, shared by both layers
            tc.tile_pool(name="eidx", bufs=2) as eidx,
            tc.tile_pool(name="eg", bufs=8) as eg,
            tc.tile_pool(name="esm", bufs=8) as esm,
            tc.tile_pool(name="eoh", bufs=4) as eoh,
            tc.tile_pool(name="ew", bufs=4) as ew,
            tc.tile_pool(name="exr", bufs=2) as exr,
            tc.tile_pool(name="eps", bufs=2, space="PSUM") as eps,
            tc.tile_pool(name="ebr", bufs=2, space="PSUM") as ebr,
            tc.tile_pool(name="etps", bufs=2, space="PSUM") as etps,
            tc.tile_pool(name="etail", bufs=2) as etail,
        ):
            # ---------- persistent DRAM intermediates
            xl1_loc = dram.tile([per, W_TAB], f32r)
            xr1_loc = dram.tile([perp, D], f32r)
            xl1_full = dram.tile([n, W_TAB], f32r, addr_space="Shared")
            h1T = dram.tile([D, perp], f32r)
            xl2_loc = dram.tile([per, W_TAB], f32r)
            xr2_loc = dram.tile([perp, D], f32r)
            xl2_full = dram.tile([n, W_TAB], f32r, addr_space="Shared")
            h2T = dram.tile([D, perp], f32r)

            # ---------- constants in SBUF
            identity = consts.tile([128, 128], f32)
            make_identity(nc, identity[:])
            id_r_t = consts.tile([128, 128], f32r)
            nc.vector.tensor_copy(id_r_t[:], identity[:])
            id_r = id_r_t[:]
            iota_f = consts.tile([128, 128], f32)
            iota_i = consts.tile([128, 128], i32)
            nc.gpsimd.iota(iota_i[:], pattern=[[1, 128]], base=0, channel_multiplier=0)
            nc.vector.tensor_copy(iota_f[:], iota_i[:])
            ones_f = consts.tile([1, 128], f32)
            nc.vector.memset(ones_f[:], 1.0)
            ones_sb = consts.tile([1, 128], f32r)
            nc.vector.tensor_copy(ones_sb[:], ones_f[:])

            if bench:
                kt = consts.tile([1, 8], i32)
                nc.sync.dma_start(kt[:], kreps[:])
                kregs = [nc.values_load(kt[0:1, j : j + 1]) for j in range(5)]

            def seg_loop(j):
                if bench:
                    return tc.For_i(0, kregs[j], 1)
                return contextlib.nullcontext()

            w1l_sb = consts.tile([in_ch + 1, D], f32r)
            nc.sync.dma_start(w1l_sb[:], w1l_aug[:])
            w1r_sb = consts.tile([in_ch + 1, D], f32r)
            nc.sync.dma_start(w1r_sb[:], w1r_aug[:])
            att1_sb = consts.tile([128, D], f32)
            nc.sync.dma_start(att1_sb[:], att1_rep[:])
            att2_sb = consts.tile([128, D], f32)
            nc.sync.dma_start(att2_sb[:], att2_rep[:])
            att15_1_sb = consts.tile([128, D], f32)
            nc.sync.dma_start(att15_1_sb[:], att15_1_rep[:])
            att15_2_sb = consts.tile([128, D], f32)
            nc.sync.dma_start(att15_2_sb[:], att15_2_rep[:])
            bias1_sb = consts.tile([128, D], f32)
            nc.sync.dma_start(bias1_sb[:], bias1_rep[:])
            bias2_sb = consts.tile([128, D], f32)
            nc.sync.dma_start(bias2_sb[:], bias2_rep[:])

            w2l_ch = []
            w2r_ch = []
            wcls_ch = []
            for k in range(4):
                t1 = consts.tile([128, D], f32r, name=f"w2l_{k}")
                nc.sync.dma_start(t1[:], w2l[k * 128 : (k + 1) * 128, :])
                w2l_ch.append(t1)
                t2 = consts.tile([128, D], f32r, name=f"w2r_{k}")
                nc.sync.dma_start(t2[:], w2r[k * 128 : (k + 1) * 128, :])
                w2r_ch.append(t2)
                t3 = consts.tile([128, 64], f32r, name=f"wcls_{k}")
                nc.sync.dma_start(t3[:], wcls[k * 128 : (k + 1) * 128, :])
                wcls_ch.append(t3)
            b2l_sb = consts.tile([1, D], f32r)
            nc.sync.dma_start(b2l_sb[:], b2l_row[:])
            b2r_sb = consts.tile([1, D], f32r)
            nc.sync.dma_start(b2r_sb[:], b2r_row[:])
            bcls_sb = consts.tile([1, 64], f32r)
            nc.sync.dma_start(bcls_sb[:], bcls_row[:])

            # zero-fill the xr tables' padded tail rows once
            if perp > per:
                zpad_f = consts.tile([128, D], f32)
                nc.vector.memset(zpad_f[:], 0.0)
                zpad = consts.tile([128, D], f32r)
                nc.vector.tensor_copy(zpad[:], zpad_f[:])
                nc.sync.dma_start(xr1_loc[per:perp, :], zpad[: perp - per, :])
                nc.sync.dma_start(xr2_loc[per:perp, :], zpad[: perp - per, :])

            # ================= stage 0: layer-1 dense transforms (local rows)
            with (
                tc.tile_pool(name="s0_in", bufs=3) as s0in,
                tc.tile_pool(name="s0_out", bufs=3) as s0out,
                seg_loop(0),
            ):
                for base, m in node_tiles:
                    lx = s0in.tile([in_ch + 1, 128], f32r, tag="lx")
                    nc.sync.dma_start(lx[:, :m], x_aug[:, base : base + m])
                    # l-branch: features + att-weighted row sums
                    psl = eps.tile([128, D], f32, space="PSUM", tag="out")
                    nc.tensor.matmul(
                        psl[:m, :], lhsT=lx[:, :m],
                        rhs=w1l_sb[:], start=True, stop=True,
                    )
                    sb = s0out.tile([128, W_TAB], f32r, tag="sb")
                    nc.scalar.copy(sb[:m, :D], psl[:m, :])
                    tmp = s0out.tile([128, D], f32, tag="tmp")
                    nc.vector.tensor_mul(tmp[:m, :], sb[:m, :D], att15_1_sb[:m, :])
                    with nc.allow_low_precision(reason="f32r is fp32-width"):
                        nc.vector.reduce_sum(
                            out=sb[:m, D:W_TAB],
                            in_=tmp[:m, :].rearrange("p (h c) -> p h c", h=HEADS),
                            axis=mybir.AxisListType.X,
                        )
                    nc.sync.dma_start(xl1_loc[base : base + m, :], sb[:m, :])
                    # r-branch: features only
                    psr = eps.tile([128, D], f32, space="PSUM", tag="out")
                    nc.tensor.matmul(
                        psr[:m, :], lhsT=lx[:, :m],
                        rhs=w1r_sb[:], start=True, stop=True,
                    )
                    sbr = s0out.tile([128, D], f32r, tag="sbr")
                    nc.scalar.copy(sbr[:m, :], psr[:m, :])
                    nc.sync.dma_start(xr1_loc[base : base + m, :], sbr[:m, :])

            nc.gpsimd.collective_compute(
                "AllGather",
                mybir.AluOpType.bypass,
                replica_groups=[list(range(n_cores))],
                ins=[xl1_loc.opt()],
                outs=[xl1_full.opt()],
            )

            # ================= edge phase (shared pools, both layers)
            def edge_phase(xl_full_ap, xr_loc_ap, att_sb, bias_sb, hT_ap, segj):
                with seg_loop(segj):
                    for b in range(blocks):
                        tb = tiles[b]
                        off = offs[b]
                        cbase = b * BLOCK
                        cols = min(BLOCK, per - cbase)
                        src_sb = eidx.tile([128, tb], i32, tag="src")
                        nc.sync.dma_start(src_sb[:], esrc[:, off : off + tb])
                        dst_sb = eidx.tile([128, tb], i32, tag="dst")
                        nc.sync.dma_start(dst_sb[:], edst[:, off : off + tb])
                        lv_sb = eidx.tile([128, tb], f32, tag="lv")
                        nc.sync.dma_start(lv_sb[:], logvalid[:, off : off + tb])
                        seg_f = eidx.tile([128, tb], f32, tag="seg")
                        nc.vector.tensor_copy(seg_f[:], dst_sb[:])
                        nc.vector.tensor_scalar_add(seg_f[:], seg_f[:], float(-cbase))

                        xr_blk = exr.tile([128, D], f32r, tag="xrb")
                        nc.sync.dma_start(xr_blk[:], xr_loc_ap[cbase : cbase + 128, :])

                        den_ps = eps.tile([128, 4], f32, space="PSUM", tag="den")
                        out_ps = eps.tile([128, D], f32, space="PSUM", tag="out")

                        for t in range(tb):
                            xg = eg.tile([128, W_TAB], f32r, tag="xg")
                            nc.gpsimd.indirect_dma_start(
                                out=xg[:],
                                out_offset=None,
                                in_=xl_full_ap,
                                in_offset=bass.IndirectOffsetOnAxis(
                                    ap=src_sb[:, t : t + 1], axis=0
                                ),
                            )
                            oh = eoh.tile([128, 128], f32r, tag="oh")
                            nc.vector.tensor_tensor(
                                out=oh[:],
                                in0=seg_f[:, t : t + 1].to_broadcast([128, 128]),
                                in1=iota_f[:],
                                op=mybir.AluOpType.is_equal,
                            )
                            ohT_ps = etps.tile([128, 128], f32r, space="PSUM", tag="tp")
                            nc.tensor.transpose(ohT_ps[:], oh[:], id_r)
                            ohT = eoh.tile([128, 128], f32r, tag="ohT")
                            nc.scalar.copy(ohT[:], ohT_ps[:])
                            xr_e = ebr.tile([128, D], f32, space="PSUM", tag="xre")
                            nc.tensor.matmul(
                                xr_e[:], lhsT=ohT[:],
                                rhs=xr_blk[:], start=True, stop=False,
                            )
                            nc.tensor.matmul(
                                xr_e[:], lhsT=id_r, rhs=xg[:, :D],
                                start=False, stop=True,
                            )
                            ab = ew.tile([128, D], f32, tag="ab")
                            nc.scalar.activation(
                                ab[:], xr_e[:], mybir.ActivationFunctionType.Abs
                            )
                            nc.vector.tensor_mul(ab[:], ab[:], att_sb[:])
                            red = esm.tile([128, 4], f32, tag="red")
                            nc.vector.reduce_sum(
                                out=red[:],
                                in_=ab[:].rearrange("p (h c) -> p h c", h=HEADS),
                                axis=mybir.AxisListType.X,
                            )
                            pre = esm.tile([128, 4], f32, tag="pre")
                            nc.vector.tensor_add(pre[:], xg[:, D:W_TAB], red[:])
                            exps = esm.tile([128, 4], f32r, tag="exps")
                            nc.scalar.activation(
                                exps[:],
                                pre[:],
                                mybir.ActivationFunctionType.Exp,
                                bias=lv_sb[:, t : t + 1],
                                scale=0.4,
                            )
                            nc.tensor.matmul(
                                den_ps[:],
                                lhsT=oh[:],
                                rhs=exps[:],
                                start=(t == 0),
                                stop=(t == tb - 1),
                            )
                            w = ew.tile([128, D], f32r, tag="w")
                            nc.vector.tensor_tensor(
                                out=w[:].rearrange("p (h c) -> p h c", h=HEADS),
                                in0=xg[:, :D].rearrange("p (h c) -> p h c", h=HEADS),
                                in1=exps[:, :, None].to_broadcast([128, HEADS, HID]),
                                op=mybir.AluOpType.mult,
                            )
                            nc.tensor.matmul(
                                out_ps[:],
                                lhsT=oh[:],
                                rhs=w[:],
                                start=(t == 0),
                                stop=(t == tb - 1),
                            )

                        # ---- block epilogue
                        den_sb = esm.tile([128, 4], f32, tag="den_sb")
                        nc.vector.tensor_copy(den_sb[:], den_ps[:])
                        recip = esm.tile([128, 4], f32, tag="recip")
                        nc.vector.reciprocal(recip[:], den_sb[:])
                        h = etail.tile([128, D], f32, tag="h")
                        nc.vector.tensor_tensor(
                            out=h[:].rearrange("p (h c) -> p h c", h=HEADS),
                            in0=out_ps[:].rearrange("p (h c) -> p h c", h=HEADS),
                            in1=recip[:, :, None].to_broadcast([128, HEADS, HID]),
                            op=mybir.AluOpType.mult,
                        )
                        nc.vector.tensor_add(h[:], h[:], bias_sb[:])
                        # ELU: relu(x) + exp(min(x,0)) - 1
                        neg = etail.tile([128, D], f32, tag="neg")
                        nc.vector.tensor_scalar_min(neg[:], h[:], 0.0)
                        expn = etail.tile([128, D], f32, tag="expn")
                        nc.scalar.activation(
                            expn[:], neg[:], mybir.ActivationFunctionType.Exp
                        )
                        pos = etail.tile([128, D], f32, tag="pos")
                        nc.scalar.activation(
                            pos[:], h[:], mybir.ActivationFunctionType.Relu
                        )
                        hf = etail.tile([128, D], f32r, tag="hf")
                        nc.vector.tensor_add(hf[:], pos[:], expn[:])
                        nc.scalar.add(hf[:], hf[:], -1.0)
                        for q in range(4):
                            tp = etps.tile([128, 128], f32r, space="PSUM", tag="tp")
                            nc.tensor.transpose(
                                tp[:], hf[:, q * 128 : (q + 1) * 128], id_r
                            )
                            tsb = etail.tile([128, 128], f32r, tag="tsb")
                            nc.scalar.copy(tsb[:], tp[:])
                            nc.sync.dma_start(
                                hT_ap[q * 128 : (q + 1) * 128, cbase : cbase + cols],
                                tsb[:, :cols],
                            )

            edge_phase(xl1_full.opt(), xr1_loc.opt(), att1_sb, bias1_sb, h1T.opt(), 1)

            # ================= stage 2: layer-2 dense transforms from h1T
            with (
                tc.tile_pool(name="s2_in", bufs=4) as s2in,
                tc.tile_pool(name="s2_out", bufs=3) as s2out,
                seg_loop(2),
            ):
                for base, m in node_tiles:
                    hts = []
                    for k in range(4):
                        ht = s2in.tile([128, 128], f32r, tag=f"ht{k}")
                        nc.sync.dma_start(
                            ht[:, :m], h1T[k * 128 : (k + 1) * 128, base : base + m]
                        )
                        hts.append(ht)
                    # l-branch
                    psl = eps.tile([128, D], f32, space="PSUM", tag="out")
                    for k in range(4):
                        nc.tensor.matmul(
                            psl[:m, :], lhsT=hts[k][:, :m],
                            rhs=w2l_ch[k][:],
                            start=(k == 0), stop=False,
                        )
                    nc.tensor.matmul(
                        psl[:m, :], lhsT=ones_sb[:, :m],
                        rhs=b2l_sb[:], start=False, stop=True,
                    )
                    sb = s2out.tile([128, W_TAB], f32r, tag="sb")
                    nc.scalar.copy(sb[:m, :D], psl[:m, :])
                    tmp = s2out.tile([128, D], f32, tag="tmp")
                    nc.vector.tensor_mul(tmp[:m, :], sb[:m, :D], att15_2_sb[:m, :])
                    with nc.allow_low_precision(reason="f32r is fp32-width"):
                        nc.vector.reduce_sum(
                            out=sb[:m, D:W_TAB],
                            in_=tmp[:m, :].rearrange("p (h c) -> p h c", h=HEADS),
                            axis=mybir.AxisListType.X,
                        )
                    nc.sync.dma_start(xl2_loc[base : base + m, :], sb[:m, :])
                    # r-branch
                    psr = eps.tile([128, D], f32, space="PSUM", tag="out")
                    for k in range(4):
                        nc.tensor.matmul(
                            psr[:m, :], lhsT=hts[k][:, :m],
                            rhs=w2r_ch[k][:],
                            start=(k == 0), stop=False,
                        )
                    nc.tensor.matmul(
                        psr[:m, :], lhsT=ones_sb[:, :m],
                        rhs=b2r_sb[:], start=False, stop=True,
                    )
                    sbr = s2out.tile([128, D], f32r, tag="sbr")
                    nc.scalar.copy(sbr[:m, :], psr[:m, :])
                    nc.sync.dma_start(xr2_loc[base : base + m, :], sbr[:m, :])

            nc.gpsimd.collective_compute(
                "AllGather",
                mybir.AluOpType.bypass,
                replica_groups=[list(range(n_cores))],
                ins=[xl2_loc.opt()],
                outs=[xl2_full.opt()],
            )

            edge_phase(xl2_full.opt(), xr2_loc.opt(), att2_sb, bias2_sb, h2T.opt(), 3)

            # ================= classifier
            with (
                tc.tile_pool(name="c_in", bufs=4) as cin,
                tc.tile_pool(name="c_out", bufs=3) as cout,
                seg_loop(4),
            ):
                for base, m in node_tiles:
                    hts = []
                    for k in range(4):
                        ht = cin.tile([128, 128], f32r, tag=f"cht{k}")
                        nc.sync.dma_start(
                            ht[:, :m], h2T[k * 128 : (k + 1) * 128, base : base + m]
                        )
                        hts.append(ht)
                    ps = eps.tile([128, 64], f32, space="PSUM", tag="out")
                    for k in range(4):
                        nc.tensor.matmul(
                            ps[:m, :], lhsT=hts[k][:, :m],
                            rhs=wcls_ch[k][:],
                            start=(k == 0), stop=False,
                        )
                    nc.tensor.matmul(
                        ps[:m, :], lhsT=ones_sb[:, :m],
                        rhs=bcls_sb[:], start=False, stop=True,
                    )
                    sb = cout.tile([128, 64], f32, tag="sb")
                    nc.scalar.copy(sb[:m, :], ps[:m, :])
                    nc.sync.dma_start(out[base : base + m, :], sb[:m, :OUT_CH])

    nc.compile()
    return nc


# ---------------------------------------------------------------- entry

_CACHE = {}


def kernel(**inputs):
    from concourse.bass_utils import run_bass_kernel_spmd

    in_maps, meta = preprocess(**inputs)
    key = (meta["tt"], tuple(meta["tiles"]))
    if key not in _CACHE:
        _CACHE[key] = build_program(meta)
    nc = _CACHE[key]
    res = run_bass_kernel_spmd(nc, in_maps, list(range(meta["n_cores"])))
    outs = [res.results[c]["out"] for c in range(meta["n_cores"])]
    return np.concatenate(outs, axis=0)


# ---------------------------------------------------------------- numpy model
# (host-side mirror of the device math, for validation in test.py)


def numpy_model(x, edge_index, w1_l, b1_l, w1_r, b1_r, att1, bias1,
                w2_l, b2_l, w2_r, b2_r, att2, bias2, w_cls, b_cls):
    x = np.asarray(x, np.float32)
    n = x.shape[0]
    ei = np.asarray(edge_index).astype(np.int64)
    loops = np.arange(n, dtype=np.int64)
    src = np.concatenate([ei[0], loops])
    dst = np.concatenate([ei[1], loops])

    def layer(h, wl, bl, wr, br, att, bias):
        att_flat = np.asarray(att, np.float32).reshape(-1)
        xl = (h @ np.asarray(wl, np.float32) + np.asarray(bl, np.float32)).astype(np.float32)
        xr = (h @ np.asarray(wr, np.float32) + np.asarray(br, np.float32)).astype(np.float32)
        adl = (xl * att_flat).reshape(n, HEADS, HID).sum(axis=2)
        z = xl[src] + xr[dst]
        abssum = (np.abs(z) * att_flat).reshape(-1, HEADS, HID).sum(axis=2)
        # adr[dst] is constant per softmax segment -> dropped (shift invariance)
        score = 0.4 * (abssum + 1.5 * adl[src])
        ex = np.exp(score)
        den = np.zeros((n, HEADS), np.float32)
        np.add.at(den, dst, ex)
        unnorm = np.zeros((n, HEADS, HID), np.float32)
        np.add.at(unnorm, dst, xl[src].reshape(-1, HEADS, HID) * ex[:, :, None])
        out = unnorm / den[:, :, None]
        out = out.reshape(n, D) + np.asarray(bias, np.float32)
        return np.where(out > 0, out, np.exp(np.minimum(out, 0)) - 1).astype(np.float32)

    h = layer(x, w1_l, b1_l, w1_r, b1_r, att1, bias1)
    h = layer(h, w2_l, b2_l, w2_r, b2_r, att2, bias2)
    return (h @ np.asarray(w_cls, np.float32) + np.asarray(b_cls, np.float32)).astype(
        np.float32
    )


# revision 19
# speedup vs baseline: 1.4331x; 1.1320x over previous
"""GATv2 (2-layer, 4 heads) + linear classifier on Trainium2, 8-core SPMD.

Sharding: nodes are partitioned contiguously across 8 cores (2500 nodes/core).
Edges are routed to the core that owns their destination node, so the
segment-softmax and scatter-add stay core-local.  The only cross-core
exchange is one AllGather per GAT layer of the source-side linear transform
table, which every core then gathers rows from by edge source id (the "halo
exchange" for cut edges).

Math decomposition (per layer, per head h with C=128 channels), using
leaky_relu(z) = 0.6 z + 0.4 |z| for slope 0.2:
  score_e = att_h . leaky_relu(z) = 0.6 * att.z + 0.4 * att.|z|
with z = xl[src] + xr[dst].  The src part of the linear term, adl[src] =
(att * xl).rowsum per head, is stored as 4 extra table columns so one gather
brings both the 512 features and the sums.  The dst part (adr[dst]) is
constant within each softmax segment and cancels, so it is dropped.
Indirect gathers are row-rate-limited (~12ns/row regardless of width), so
the xr[dst] side is never gathered: each 128-dst block loads its xr rows
with one direct DMA and broadcasts them to edges on the tensor engine using
the transposed one-hot (edge -> segment) matrix.
Softmax max-subtraction is skipped (scores are in [-6, 6], exact in fp32).
Per block, one-hot matmuls accumulate the exp-weight segment sum ("denom")
and the exp-weighted feature segment sum into PSUM; the block epilogue
divides by denom, adds the output bias, applies ELU, and stores the block
transposed (feature-major) for the next layer's matmuls.
"""

import numpy as np

N_NODES = 20000
IN_CH = 55
HID = 128
HEADS = 4
D = HID * HEADS  # 512
OUT_CH = 49
NCORES = 8
PER_CORE = N_NODES // NCORES  # 2500
BLOCK = 128
NEG_SLOPE = 0.2
W_TAB = D + HEADS  # 516: features + per-head att-weighted row sums


# ---------------------------------------------------------------- host prep


def _plan_edges(src, dst, n_nodes, per_core, n_cores):
    """Route edges to (core, block) by dst; pad each block to a multiple of
    128 edge slots, uniformly across cores (SPMD program must be identical).
    """
    blocks_per_core = (per_core + BLOCK - 1) // BLOCK
    core = dst // per_core
    dst_local = dst - core * per_core
    blk = dst_local // BLOCK

    counts = np.zeros((n_cores, blocks_per_core), dtype=np.int64)
    np.add.at(counts, (core, blk), 1)
    tiles = np.maximum(1, -(-counts.max(axis=0) // 128))
    offs = np.concatenate([[0], np.cumsum(tiles)])[:-1]
    tt = int(tiles.sum())

    esrc = np.zeros((n_cores, 128, tt), dtype=np.int32)
    edst = np.zeros((n_cores, 128, tt), dtype=np.int32)
    lv = np.full((n_cores, 128, tt), -1e30, dtype=np.float32)

    order = np.lexsort((blk, core))
    src_s, dstl_s = src[order], dst_local[order]
    key = core[order] * blocks_per_core + blk[order]
    bounds = np.searchsorted(key, np.arange(n_cores * blocks_per_core + 1))
    for c in range(n_cores):
        for b in range(blocks_per_core):
            k = c * blocks_per_core + b
            lo, hi = bounds[k], bounds[k + 1]
            cnt = hi - lo
            nslots = int(tiles[b]) * 128
            s = np.zeros(nslots, dtype=np.int32)
            d_ = np.zeros(nslots, dtype=np.int32)
            v = np.full(nslots, -1e30, dtype=np.float32)
            s[:cnt] = src_s[lo:hi]
            d_[:cnt] = dstl_s[lo:hi]
            d_[cnt:] = b * BLOCK  # dummy slots: stay inside this block
            v[:cnt] = 0.0
            o = int(offs[b])
            t = int(tiles[b])
            esrc[c][:, o : o + t] = s.reshape(t, 128).T
            edst[c][:, o : o + t] = d_.reshape(t, 128).T
            lv[c][:, o : o + t] = v.reshape(t, 128).T
    return dict(
        tiles=[int(t) for t in tiles],
        offs=[int(o) for o in offs],
        tt=tt,
        blocks=blocks_per_core,
        esrc=esrc,
        edst=edst,
        logvalid=lv,
    )


def _bf16(a):
    import ml_dtypes
    return np.asarray(a, np.float32).astype(ml_dtypes.bfloat16)


def preprocess(x, edge_index, w1_l, b1_l, w1_r, b1_r, att1, bias1,
               w2_l, b2_l, w2_r, b2_r, att2, bias2, w_cls, b_cls,
               n_cores=NCORES):
    x = np.asarray(x, np.float32)
    n = x.shape[0]
    per = n // n_cores
    ei = np.asarray(edge_index).astype(np.int64)
    loops = np.arange(n, dtype=np.int64)
    src = np.concatenate([ei[0], loops])
    dst = np.concatenate([ei[1], loops])

    att1_flat = np.asarray(att1, np.float32).reshape(-1)
    att2_flat = np.asarray(att2, np.float32).reshape(-1)

    plan = _plan_edges(src, dst, n, per, n_cores)

    in_ch = x.shape[1]
    aug = lambda w, b: np.concatenate(
        [np.asarray(w, np.float32), np.asarray(b, np.float32)[None, :]], axis=0
    )
    rep = lambda v: np.broadcast_to(
        np.asarray(v, np.float32)[None, :], (128, v.shape[0])
    ).copy()

    shared = {
        "w1l_aug": aug(w1_l, b1_l),
        "w1r_aug": aug(w1_r, b1_r),
        "w2l": np.asarray(w2_l, np.float32),
        "w2r": np.asarray(w2_r, np.float32),
        "b2l_row": np.asarray(b2_l, np.float32)[None, :],
        "b2r_row": np.asarray(b2_r, np.float32)[None, :],
        "wcls": np.pad(np.asarray(w_cls, np.float32), ((0, 0), (0, 64 - OUT_CH))),
        "bcls_row": np.pad(np.asarray(b_cls, np.float32), (0, 64 - OUT_CH))[None, :],
        "att1_rep": rep(att1_flat),
        "att2_rep": rep(att2_flat),
        "att15_1_rep": rep(1.5 * att1_flat),
        "att15_2_rep": rep(1.5 * att2_flat),
        "bias1_rep": rep(np.asarray(bias1, np.float32)),
        "bias2_rep": rep(np.asarray(bias2, np.float32)),
    }
    in_maps = []
    for c in range(n_cores):
        xa = np.concatenate(
            [x[c * per : (c + 1) * per].T, np.ones((1, per), np.float32)], axis=0
        )
        m = dict(shared)
        m["x_aug"] = np.ascontiguousarray(xa)
        m["esrc"] = plan["esrc"][c]
        m["edst"] = plan["edst"][c]
        m["logvalid"] = plan["logvalid"][c]
        in_maps.append(m)
    meta = dict(
        n=n, per=per, in_ch=in_ch, tiles=plan["tiles"], offs=plan["offs"],
        tt=plan["tt"], blocks=plan["blocks"], n_cores=n_cores,
    )
    return in_maps, meta


# ---------------------------------------------------------------- device


def build_program(meta, bench=False, debug_outs=False):
    import contextlib
    import concourse.bass as bass
    import concourse.tile as tile
    import concourse.mybir as mybir
    from concourse import bacc
    from concourse.masks import make_identity

    f32 = mybir.dt.float32
    bf16 = mybir.dt.bfloat16
    f32r = mybir.dt.float32r
    i32 = mybir.dt.int32

    n = meta["n"]
    per = meta["per"]
    in_ch = meta["in_ch"]
    tiles = meta["tiles"]
    offs = meta["offs"]
    tt = meta["tt"]
    blocks = meta["blocks"]
    n_cores = meta["n_cores"]
    perp = blocks * BLOCK  # xr tables padded to full blocks

    nt_full, nt_rem = divmod(per, 128)
    node_tiles = [(i * 128, 128) for i in range(nt_full)]
    if nt_rem:
        node_tiles.append((nt_full * 128, nt_rem))

    nc = bacc.Bacc("TRN2", target_bir_lowering=False, debug=False, num_devices=n_cores)

    # register a -1.0 const AP so scalar.add(x, -1.0) lowers on the ACT engine
    _cm1 = nc.alloc_sbuf_tensor("const-float32-neg1", [128, 1], f32)
    nc.gpsimd.memset(_cm1.ap(), -1.0)
    nc.const_aps.aps[(f32, -1.0)] = _cm1.ap()

    def din(name, shape, dt=f32):
        return nc.dram_tensor(name, shape, dt, kind="ExternalInput").ap()

    x_aug = din("x_aug", [in_ch + 1, per], f32r)
    w1l_aug = din("w1l_aug", [in_ch + 1, D], f32r)
    w1r_aug = din("w1r_aug", [in_ch + 1, D], f32r)
    w2l = din("w2l", [D, D], f32r)
    w2r = din("w2r", [D, D], f32r)
    b2l_row = din("b2l_row", [1, D], f32r)
    b2r_row = din("b2r_row", [1, D], f32r)
    wcls = din("wcls", [D, 64], f32r)
    bcls_row = din("bcls_row", [1, 64], f32r)
    att1_rep = din("att1_rep", [128, D])
    att2_rep = din("att2_rep", [128, D])
    att15_1_rep = din("att15_1_rep", [128, D])
    att15_2_rep = din("att15_2_rep", [128, D])
    bias1_rep = din("bias1_rep", [128, D])
    bias2_rep = din("bias2_rep", [128, D])
    esrc = din("esrc", [128, tt], i32)
    edst = din("edst", [128, tt], i32)
    logvalid = din("logvalid", [128, tt])
    kreps = din("kreps", [1, 8], i32) if bench else None

    out = nc.dram_tensor("out", [per, OUT_CH], f32, kind="ExternalOutput").ap()
    dbg_h1T = (
        nc.dram_tensor("dbg_h1T", [D, blocks * BLOCK], f32, kind="ExternalOutput").ap()
        if debug_outs
        else None
    )
    dbg_xl2 = (
        nc.dram_tensor("dbg_xl2", [per, W_TAB], f32, kind="ExternalOutput").ap()
        if debug_outs
        else None
    )
    dbg_h2T = (
        nc.dram_tensor("dbg_h2T", [D, blocks * BLOCK], f32, kind="ExternalOutput").ap()
        if debug_outs
        else None
    )

    with tile.TileContext(nc) as tc:
        with (
            tc.tile_pool(name="dram", bufs=1, space="DRAM") as dram,
            tc.tile_pool(name="consts", bufs=1) as consts,
            # ---- edge-phase pools, shared by both layers
            tc.tile_pool(name="eidx", bufs=2) as eidx,
            tc.tile_pool(name="eg", bufs=8) as eg,
            tc.tile_pool(name="esm", bufs=8) as esm,
            tc.tile_pool(name="eoh", bufs=4) as eoh,
            tc.tile_pool(name="ew", bufs=4) as ew,
            tc.tile_pool(name="exr", bufs=2) as exr,
            tc.tile_pool(name="eps", bufs=2, space="PSUM") as eps,
            tc.tile_pool(name="ebr", bufs=2, space="PSUM") as ebr,
            tc.tile_pool(name="etps", bufs=2, space="PSUM") as etps,
            tc.tile_pool(name="etail", bufs=2) as etail,
        ):
            # ---------- persistent DRAM intermediates
            xl1_loc = dram.tile([per, W_TAB], f32r)
            xr1_loc = dram.tile([perp, D], f32r)
            xl1_full = dram.tile([n, W_TAB], f32r, addr_space="Shared")
            h1T = dram.tile([D, perp], f32r)
            xl2_loc = dram.tile([per, W_TAB], f32r)
            xr2_loc = dram.tile([perp, D], f32r)
            xl2_full = dram.tile([n, W_TAB], f32r, addr_space="Shared")
            h2T = dram.tile([D, perp], f32r)

            # ---------- constants in SBUF
            identity = consts.tile([128, 128], f32)
            make_identity(nc, identity[:])
            id_r_t = consts.tile([128, 128], f32r)
            nc.vector.tensor_copy(id_r_t[:], identity[:])
            id_r = id_r_t[:]
            iota_f = consts.tile([128, 128], f32)
            iota_i = consts.tile([128, 128], i32)
            nc.gpsimd.iota(iota_i[:], pattern=[[1, 128]], base=0, channel_multiplier=0)
            nc.vector.tensor_copy(iota_f[:], iota_i[:])
            ones_f = consts.tile([1, 128], f32)
            nc.vector.memset(ones_f[:], 1.0)
            ones_sb = consts.tile([1, 128], f32r)
            nc.vector.tensor_copy(ones_sb[:], ones_f[:])

            if bench:
                kt = consts.tile([1, 8], i32)
                nc.sync.dma_start(kt[:], kreps[:])
                kregs = [nc.values_load(kt[0:1, j : j + 1]) for j in range(5)]

            def seg_loop(j):
                if bench:
                    return tc.For_i(0, kregs[j], 1)
                return contextlib.nullcontext()

            w1l_sb = consts.tile([in_ch + 1, D], f32r)
            nc.sync.dma_start(w1l_sb[:], w1l_aug[:])
            w1r_sb = consts.tile([in_ch + 1, D], f32r)
            nc.sync.dma_start(w1r_sb[:], w1r_aug[:])
            att1_sb = consts.tile([128, D], f32)
            nc.sync.dma_start(att1_sb[:], att1_rep[:])
            att2_sb = consts.tile([128, D], f32)
            nc.sync.dma_start(att2_sb[:], att2_rep[:])
            att15_1_sb = consts.tile([128, D], f32)
            nc.sync.dma_start(att15_1_sb[:], att15_1_rep[:])
            att15_2_sb = consts.tile([128, D], f32)
            nc.sync.dma_start(att15_2_sb[:], att15_2_rep[:])
            bias1_sb = consts.tile([128, D], f32)
            nc.sync.dma_start(bias1_sb[:], bias1_rep[:])
            bias2_sb = consts.tile([128, D], f32)
            nc.sync.dma_start(bias2_sb[:], bias2_rep[:])

            w2l_ch = []
            w2r_ch = []
            wcls_ch = []
            for k in range(4):
                t1 = consts.tile([128, D], f32r, name=f"w2l_{k}")
                nc.sync.dma_start(t1[:], w2l[k * 128 : (k + 1) * 128, :])
                w2l_ch.append(t1)
                t2 = consts.tile([128, D], f32r, name=f"w2r_{k}")
                nc.sync.dma_start(t2[:], w2r[k * 128 : (k + 1) * 128, :])
                w2r_ch.append(t2)
                t3 = consts.tile([128, 64], f32r, name=f"wcls_{k}")
                nc.sync.dma_start(t3[:], wcls[k * 128 : (k + 1) * 128, :])
                wcls_ch.append(t3)
            b2l_sb = consts.tile([1, D], f32r)
            nc.sync.dma_start(b2l_sb[:], b2l_row[:])
            b2r_sb = consts.tile([1, D], f32r)
            nc.sync.dma_start(b2r_sb[:], b2r_row[:])
            bcls_sb = consts.tile([1, 64], f32r)
            nc.sync.dma_start(bcls_sb[:], bcls_row[:])

            # zero-fill the xr tables' padded tail rows once
            if perp > per:
                zpad_f = consts.tile([128, D], f32)
                nc.vector.memset(zpad_f[:], 0.0)
                zpad = consts.tile([128, D], f32r)
                nc.vector.tensor_copy(zpad[:], zpad_f[:])
                nc.sync.dma_start(xr1_loc[per:perp, :], zpad[: perp - per, :])
                nc.sync.dma_start(xr2_loc[per:perp, :], zpad[: perp - per, :])

            # ================= stage 0: layer-1 dense transforms (local rows)
            with (
                tc.tile_pool(name="s0_in", bufs=3) as s0in,
                tc.tile_pool(name="s0_out", bufs=3) as s0out,
                seg_loop(0),
            ):
                for base, m in node_tiles:
                    lx = s0in.tile([in_ch + 1, 128], f32r, tag="lx")
                    nc.sync.dma_start(lx[:, :m], x_aug[:, base : base + m])
                    # l-branch: features + att-weighted row sums
                    psl = eps.tile([128, D], f32, space="PSUM", tag="out")
                    nc.tensor.matmul(
                        psl[:m, :], lhsT=lx[:, :m],
                        rhs=w1l_sb[:], start=True, stop=True,
                    )
                    sb = s0out.tile([128, W_TAB], f32r, tag="sb")
                    nc.scalar.copy(sb[:m, :D], psl[:m, :])
                    tmp = s0out.tile([128, D], f32, tag="tmp")
                    nc.vector.tensor_mul(tmp[:m, :], sb[:m, :D], att15_1_sb[:m, :])
                    with nc.allow_low_precision(reason="f32r is fp32-width"):
                        nc.vector.reduce_sum(
                            out=sb[:m, D:W_TAB],
                            in_=tmp[:m, :].rearrange("p (h c) -> p h c", h=HEADS),
                            axis=mybir.AxisListType.X,
                        )
                    nc.sync.dma_start(xl1_loc[base : base + m, :], sb[:m, :])
                    # r-branch: features only
                    psr = eps.tile([128, D], f32, space="PSUM", tag="out")
                    nc.tensor.matmul(
                        psr[:m, :], lhsT=lx[:, :m],
                        rhs=w1r_sb[:], start=True, stop=True,
                    )
                    sbr = s0out.tile([128, D], f32r, tag="sbr")
                    nc.scalar.copy(sbr[:m, :], psr[:m, :])
                    nc.sync.dma_start(xr1_loc[base : base + m, :], sbr[:m, :])

            nc.gpsimd.collective_compute(
                "AllGather",
                mybir.AluOpType.bypass,
                replica_groups=[list(range(n_cores))],
                ins=[xl1_loc.opt()],
                outs=[xl1_full.opt()],
            )

            # ================= edge phase (shared pools, both layers)
            def edge_phase(xl_full_ap, xr_loc_ap, att_sb, bias_sb, hT_ap, segj):
                with seg_loop(segj):
                    for b in range(blocks):
                        tb = tiles[b]
                        off = offs[b]
                        cbase = b * BLOCK
                        cols = min(BLOCK, per - cbase)
                        src_sb = eidx.tile([128, tb], i32, tag="src")
                        nc.sync.dma_start(src_sb[:], esrc[:, off : off + tb])
                        dst_sb = eidx.tile([128, tb], i32, tag="dst")
                        nc.sync.dma_start(dst_sb[:], edst[:, off : off + tb])
                        lv_sb = eidx.tile([128, tb], f32, tag="lv")
                        nc.sync.dma_start(lv_sb[:], logvalid[:, off : off + tb])
                        seg_f = eidx.tile([128, tb], f32, tag="seg")
                        nc.vector.tensor_copy(seg_f[:], dst_sb[:])
                        nc.vector.tensor_scalar_add(seg_f[:], seg_f[:], float(-cbase))

                        xr_blk = exr.tile([128, D], f32r, tag="xrb")
                        nc.sync.dma_start(xr_blk[:], xr_loc_ap[cbase : cbase + 128, :])

                        den_ps = eps.tile([128, 4], f32, space="PSUM", tag="den")
                        out_ps = eps.tile([128, D], f32, space="PSUM", tag="out")

                        for t in range(tb):
                            xg = eg.tile([128, W_TAB], f32r, tag="xg")
                            nc.gpsimd.indirect_dma_start(
                                out=xg[:],
                                out_offset=None,
                                in_=xl_full_ap,
                                in_offset=bass.IndirectOffsetOnAxis(
                                    ap=src_sb[:, t : t + 1], axis=0
                                ),
                            )
                            oh = eoh.tile([128, 128], f32r, tag="oh")
                            nc.vector.tensor_tensor(
                                out=oh[:],
                                in0=seg_f[:, t : t + 1].to_broadcast([128, 128]),
                                in1=iota_f[:],
                                op=mybir.AluOpType.is_equal,
                            )
                            ohT_ps = etps.tile([128, 128], f32r, space="PSUM", tag="tp")
                            nc.tensor.transpose(ohT_ps[:], oh[:], id_r)
                            ohT = eoh.tile([128, 128], f32r, tag="ohT")
                            nc.scalar.copy(ohT[:], ohT_ps[:])
                            xr_e = ebr.tile([128, D], f32, space="PSUM", tag="xre")
                            nc.tensor.matmul(
                                xr_e[:], lhsT=ohT[:],
                                rhs=xr_blk[:], start=True, stop=False,
                            )
                            nc.tensor.matmul(
                                xr_e[:], lhsT=id_r, rhs=xg[:, :D],
                                start=False, stop=True,
                            )
                            ab = ew.tile([128, D], f32, tag="ab")
                            nc.scalar.activation(
                                ab[:], xr_e[:], mybir.ActivationFunctionType.Abs
                            )
                            nc.vector.tensor_mul(ab[:], ab[:], att_sb[:])
                            red = esm.tile([128, 4], f32, tag="red")
                            nc.vector.reduce_sum(
                                out=red[:],
                                in_=ab[:].rearrange("p (h c) -> p h c", h=HEADS),
                                axis=mybir.AxisListType.X,
                            )
                            pre = esm.tile([128, 4], f32, tag="pre")
                            nc.vector.tensor_add(pre[:], xg[:, D:W_TAB], red[:])
                            exps = esm.tile([128, 4], f32r, tag="exps")
                            nc.scalar.activation(
                                exps[:],
                                pre[:],
                                mybir.ActivationFunctionType.Exp,
                                bias=lv_sb[:, t : t + 1],
                                scale=0.4,
                            )
                            nc.tensor.matmul(
                                den_ps[:],
                                lhsT=oh[:],
                                rhs=exps[:],
                                start=(t == 0),
                                stop=(t == tb - 1),
                            )
                            w = ew.tile([128, D], f32r, tag="w")
                            nc.vector.tensor_tensor(
                                out=w[:].rearrange("p (h c) -> p h c", h=HEADS),
                                in0=xg[:, :D].rearrange("p (h c) -> p h c", h=HEADS),
                                in1=exps[:, :, None].to_broadcast([128, HEADS, HID]),
                                op=mybir.AluOpType.mult,
                            )
                            nc.tensor.matmul(
                                out_ps[:],
                                lhsT=oh[:],
                                rhs=w[:],
                                start=(t == 0),
                                stop=(t == tb - 1),
                            )

                        # ---- block epilogue
                        den_sb = esm.tile([128, 4], f32, tag="den_sb")
                        nc.vector.tensor_copy(den_sb[:], den_ps[:])
                        recip = esm.tile([128, 4], f32, tag="recip")
                        nc.vector.reciprocal(recip[:], den_sb[:])
                        h = etail.tile([128, D], f32, tag="h")
                        nc.vector.tensor_tensor(
                            out=h[:].rearrange("p (h c) -> p h c", h=HEADS),
                            in0=out_ps[:].rearrange("p (h c) -> p h c", h=HEADS),
                            in1=recip[:, :, None].to_broadcast([128, HEADS, HID]),
                            op=mybir.AluOpType.mult,
                        )
                        nc.vector.tensor_add(h[:], h[:], bias_sb[:])
                        # ELU: relu(x) + exp(min(x,0)) - 1
                        neg = etail.tile([128, D], f32, tag="neg")
                        nc.vector.tensor_scalar_min(neg[:], h[:], 0.0)
                        expn = etail.tile([128, D], f32, tag="expn")
                        nc.scalar.activation(
                            expn[:], neg[:], mybir.ActivationFunctionType.Exp
                        )
                        pos = etail.tile([128, D], f32, tag="pos")
                        nc.scalar.activation(
                            pos[:], h[:], mybir.ActivationFunctionType.Relu
                        )
                        hf = etail.tile([128, D], f32r, tag="hf")
                        nc.vector.tensor_add(hf[:], pos[:], expn[:])
                        nc.scalar.add(hf[:], hf[:], -1.0)
                        for q in range(4):
                            tp = etps.tile([128, 128], f32r, space="PSUM", tag="tp")
                            nc.tensor.transpose(
                                tp[:], hf[:, q * 128 : (q + 1) * 128], id_r
                            )
                            tsb = etail.tile([128, 128], f32r, tag="tsb")
                            nc.scalar.copy(tsb[:], tp[:])
                            nc.sync.dma_start(
                                hT_ap[q * 128 : (q + 1) * 128, cbase : cbase + cols],
                                tsb[:, :cols],
                            )

            edge_phase(xl1_full.opt(), xr1_loc.opt(), att1_sb, bias1_sb, h1T.opt(), 1)

            # ================= stage 2: layer-2 dense transforms from h1T
            with (
                tc.tile_pool(name="s2_in", bufs=4) as s2in,
                tc.tile_pool(name="s2_out", bufs=3) as s2out,
                seg_loop(2),
            ):
                for base, m in node_tiles:
                    hts = []
                    for k in range(4):
                        ht = s2in.tile([128, 128], f32r, tag=f"ht{k}")
                        nc.sync.dma_start(
                            ht[:, :m], h1T[k * 128 : (k + 1) * 128, base : base + m]
                        )
                        hts.append(ht)
                    # l-branch
                    psl = eps.tile([128, D], f32, space="PSUM", tag="out")
                    for k in range(4):
                        nc.tensor.matmul(
                            psl[:m, :], lhsT=hts[k][:, :m],
                            rhs=w2l_ch[k][:],
                            start=(k == 0), stop=False,
                        )
                    nc.tensor.matmul(
                        psl[:m, :], lhsT=ones_sb[:, :m],
                        rhs=b2l_sb[:], start=False, stop=True,
                    )
                    sb = s2out.tile([128, W_TAB], f32r, tag="sb")
                    nc.scalar.copy(sb[:m, :D], psl[:m, :])
                    tmp = s2out.tile([128, D], f32, tag="tmp")
                    nc.vector.tensor_mul(tmp[:m, :], sb[:m, :D], att15_2_sb[:m, :])
                    with nc.allow_low_precision(reason="f32r is fp32-width"):
                        nc.vector.reduce_sum(
                            out=sb[:m, D:W_TAB],
                            in_=tmp[:m, :].rearrange("p (h c) -> p h c", h=HEADS),
                            axis=mybir.AxisListType.X,
                        )
                    nc.sync.dma_start(xl2_loc[base : base + m, :], sb[:m, :])
                    # r-branch
                    psr = eps.tile([128, D], f32, space="PSUM", tag="out")
                    for k in range(4):
                        nc.tensor.matmul(
                            psr[:m, :], lhsT=hts[k][:, :m],
                            rhs=w2r_ch[k][:],
                            start=(k == 0), stop=False,
                        )
                    nc.tensor.matmul(
                        psr[:m, :], lhsT=ones_sb[:, :m],
                        rhs=b2r_sb[:], start=False, stop=True,
                    )
                    sbr = s2out.tile([128, D], f32r, tag="sbr")
                    nc.scalar.copy(sbr[:m, :], psr[:m, :])
                    nc.sync.dma_start(xr2_loc[base : base + m, :], sbr[:m, :])

            nc.gpsimd.collective_compute(
                "AllGather",
                mybir.AluOpType.bypass,
                replica_groups=[list(range(n_cores))],
                ins=[xl2_loc.opt()],
                outs=[xl2_full.opt()],
            )

            edge_phase(xl2_full.opt(), xr2_loc.opt(), att2_sb, bias2_sb, h2T.opt(), 3)

            # ================= classifier
            with (
                tc.tile_pool(name="c_in", bufs=4) as cin,
                tc.tile_pool(name="c_out", bufs=3) as cout,
                seg_loop(4),
            ):
                for base, m in node_tiles:
                    hts = []
                    for k in range(4):
                        ht = cin.tile([128, 128], f32r, tag=f"cht{k}")
                        nc.sync.dma_start(
                            ht[:, :m], h2T[k * 128 : (k + 1) * 128, base : base + m]
                        )
                        hts.append(ht)
                    ps = eps.tile([128, 64], f32, space="PSUM", tag="out")
                    for k in range(4):
                        nc.tensor.matmul(
                            ps[:m, :], lhsT=hts[k][:, :m],
                            rhs=wcls_ch[k][:],
                            start=(k == 0), stop=False,
                        )
                    nc.tensor.matmul(
                        ps[:m, :], lhsT=ones_sb[:, :m],
                        rhs=bcls_sb[:], start=False, stop=True,
                    )
                    sb = cout.tile([128, 64], f32, tag="sb")
                    nc.scalar.copy(sb[:m, :], ps[:m, :])
                    nc.sync.dma_start(out[base : base + m, :], sb[:m, :OUT_CH])

    nc.compile()
    return nc


# ---------------------------------------------------------------- entry

_CACHE = {}


def kernel(**inputs):
    from concourse.bass_utils import run_bass_kernel_spmd

    in_maps, meta = preprocess(**inputs)
    key = (meta["tt"], tuple(meta["tiles"]))
    if key not in _CACHE:
        _CACHE[key] = build_program(meta)
    nc = _CACHE[key]
    res = run_bass_kernel_spmd(nc, in_maps, list(range(meta["n_cores"])))
    outs = [res.results[c]["out"] for c in range(meta["n_cores"])]
    return np.concatenate(outs, axis=0)


# ---------------------------------------------------------------- numpy model
# (host-side mirror of the device math, for validation in test.py)


def numpy_model(x, edge_index, w1_l, b1_l, w1_r, b1_r, att1, bias1,
                w2_l, b2_l, w2_r, b2_r, att2, bias2, w_cls, b_cls):
    x = np.asarray(x, np.float32)
    n = x.shape[0]
    ei = np.asarray(edge_index).astype(np.int64)
    loops = np.arange(n, dtype=np.int64)
    src = np.concatenate([ei[0], loops])
    dst = np.concatenate([ei[1], loops])

    def layer(h, wl, bl, wr, br, att, bias):
        att_flat = np.asarray(att, np.float32).reshape(-1)
        xl = (h @ np.asarray(wl, np.float32) + np.asarray(bl, np.float32)).astype(np.float32)
        xr = (h @ np.asarray(wr, np.float32) + np.asarray(br, np.float32)).astype(np.float32)
        adl = (xl * att_flat).reshape(n, HEADS, HID).sum(axis=2)
        z = xl[src] + xr[dst]
        abssum = (np.abs(z) * att_flat).reshape(-1, HEADS, HID).sum(axis=2)
        # adr[dst] is constant per softmax segment -> dropped (shift invariance)
        score = 0.4 * (abssum + 1.5 * adl[src])
        ex = np.exp(score)
        den = np.zeros((n, HEADS), np.float32)
        np.add.at(den, dst, ex)
        unnorm = np.zeros((n, HEADS, HID), np.float32)
        np.add.at(unnorm, dst, xl[src].reshape(-1, HEADS, HID) * ex[:, :, None])
        out = unnorm / den[:, :, None]
        out = out.reshape(n, D) + np.asarray(bias, np.float32)
        return np.where(out > 0, out, np.exp(np.minimum(out, 0)) - 1).astype(np.float32)

    h = layer(x, w1_l, b1_l, w1_r, b1_r, att1, bias1)
    h = layer(h, w2_l, b2_l, w2_r, b2_r, att2, bias2)
    return (h @ np.asarray(w_cls, np.float32) + np.asarray(b_cls, np.float32)).astype(
        np.float32
    )


# revision 25
# speedup vs baseline: 1.4508x; 1.0124x over previous
"""GATv2 (2-layer, 4 heads) + linear classifier on Trainium2, 8-core SPMD.

Sharding: nodes are partitioned contiguously across 8 cores (2500 nodes/core).
Edges are routed to the core that owns their destination node, so the
segment-softmax and scatter-add stay core-local.  The only cross-core
exchange is one AllGather per GAT layer of the source-side linear transform
table, which every core then gathers rows from by edge source id (the "halo
exchange" for cut edges).

Math decomposition (per layer, per head h with C=128 channels), using
leaky_relu(z) = 0.6 z + 0.4 |z| for slope 0.2:
  score_e = att_h . leaky_relu(z) = 0.6 * att.z + 0.4 * att.|z|
with z = xl[src] + xr[dst].  The src part of the linear term, adl[src] =
(att * xl).rowsum per head, is stored as 4 extra table columns so one gather
brings both the 512 features and the sums.  The dst part (adr[dst]) is
constant within each softmax segment and cancels, so it is dropped.
Indirect gathers are row-rate-limited (~12ns/row regardless of width), so
the xr[dst] side is never gathered: each 128-dst block loads its xr rows
with one direct DMA and broadcasts them to edges on the tensor engine using
the transposed one-hot (edge -> segment) matrix.
Softmax max-subtraction is skipped (scores are in [-6, 6], exact in fp32).
Per block, one-hot matmuls accumulate the exp-weight segment sum ("denom")
and the exp-weighted feature segment sum into PSUM; the block epilogue
divides by denom, adds the output bias, applies ELU, and stores the block
transposed (feature-major) for the next layer's matmuls.
"""

import numpy as np

N_NODES = 20000
IN_CH = 55
HID = 128
HEADS = 4
D = HID * HEADS  # 512
OUT_CH = 49
NCORES = 8
PER_CORE = N_NODES // NCORES  # 2500
BLOCK = 128
NEG_SLOPE = 0.2
W_TAB = D + HEADS  # 516: features + per-head att-weighted row sums


# ---------------------------------------------------------------- host prep


def _plan_edges(src, dst, n_nodes, per_core, n_cores):
    """Route edges to (core, block) by dst; pad each block to a multiple of
    128 edge slots, uniformly across cores (SPMD program must be identical).
    """
    blocks_per_core = (per_core + BLOCK - 1) // BLOCK
    core = dst // per_core
    dst_local = dst - core * per_core
    blk = dst_local // BLOCK

    counts = np.zeros((n_cores, blocks_per_core), dtype=np.int64)
    np.add.at(counts, (core, blk), 1)
    tiles = np.maximum(1, -(-counts.max(axis=0) // 128))
    offs = np.concatenate([[0], np.cumsum(tiles)])[:-1]
    tt = int(tiles.sum())

    esrc = np.zeros((n_cores, 128, tt), dtype=np.int32)
    edst = np.zeros((n_cores, 128, tt), dtype=np.int32)
    lv = np.full((n_cores, 128, tt), -1e30, dtype=np.float32)

    order = np.lexsort((blk, core))
    src_s, dstl_s = src[order], dst_local[order]
    key = core[order] * blocks_per_core + blk[order]
    bounds = np.searchsorted(key, np.arange(n_cores * blocks_per_core + 1))
    for c in range(n_cores):
        for b in range(blocks_per_core):
            k = c * blocks_per_core + b
            lo, hi = bounds[k], bounds[k + 1]
            cnt = hi - lo
            nslots = int(tiles[b]) * 128
            s = np.zeros(nslots, dtype=np.int32)
            d_ = np.zeros(nslots, dtype=np.int32)
            v = np.full(nslots, -1e30, dtype=np.float32)
            s[:cnt] = src_s[lo:hi]
            d_[:cnt] = dstl_s[lo:hi]
            d_[cnt:] = b * BLOCK  # dummy slots: stay inside this block
            v[:cnt] = 0.0
            o = int(offs[b])
            t = int(tiles[b])
            esrc[c][:, o : o + t] = s.reshape(t, 128).T
            edst[c][:, o : o + t] = d_.reshape(t, 128).T
            lv[c][:, o : o + t] = v.reshape(t, 128).T
    return dict(
        tiles=[int(t) for t in tiles],
        offs=[int(o) for o in offs],
        tt=tt,
        blocks=blocks_per_core,
        esrc=esrc,
        edst=edst,
        logvalid=lv,
    )


def _bf16(a):
    import ml_dtypes
    return np.asarray(a, np.float32).astype(ml_dtypes.bfloat16)


def preprocess(x, edge_index, w1_l, b1_l, w1_r, b1_r, att1, bias1,
               w2_l, b2_l, w2_r, b2_r, att2, bias2, w_cls, b_cls,
               n_cores=NCORES):
    x = np.asarray(x, np.float32)
    n = x.shape[0]
    per = n // n_cores
    ei = np.asarray(edge_index).astype(np.int64)
    loops = np.arange(n, dtype=np.int64)
    src = np.concatenate([ei[0], loops])
    dst = np.concatenate([ei[1], loops])

    att1_flat = np.asarray(att1, np.float32).reshape(-1)
    att2_flat = np.asarray(att2, np.float32).reshape(-1)

    plan = _plan_edges(src, dst, n, per, n_cores)

    in_ch = x.shape[1]
    aug = lambda w, b: np.concatenate(
        [np.asarray(w, np.float32), np.asarray(b, np.float32)[None, :]], axis=0
    )
    rep = lambda v: np.broadcast_to(
        np.asarray(v, np.float32)[None, :], (128, v.shape[0])
    ).copy()

    shared = {
        "w1l_aug": aug(w1_l, b1_l),
        "w1r_aug": aug(w1_r, b1_r),
        "w2l": np.asarray(w2_l, np.float32),
        "w2r": np.asarray(w2_r, np.float32),
        "b2l_row": np.asarray(b2_l, np.float32)[None, :],
        "b2r_row": np.asarray(b2_r, np.float32)[None, :],
        "wcls": np.pad(np.asarray(w_cls, np.float32), ((0, 0), (0, 64 - OUT_CH))),
        "bcls_row": np.pad(np.asarray(b_cls, np.float32), (0, 64 - OUT_CH))[None, :],
        "att1_rep": rep(att1_flat),
        "att2_rep": rep(att2_flat),
        "att15_1_rep": rep(1.5 * att1_flat),
        "att15_2_rep": rep(1.5 * att2_flat),
        "bias1_rep": rep(np.asarray(bias1, np.float32)),
        "bias2_rep": rep(np.asarray(bias2, np.float32)),
    }
    in_maps = []
    for c in range(n_cores):
        xa = np.concatenate(
            [x[c * per : (c + 1) * per].T, np.ones((1, per), np.float32)], axis=0
        )
        m = dict(shared)
        m["x_aug"] = np.ascontiguousarray(xa)
        m["esrc"] = plan["esrc"][c]
        m["edst"] = plan["edst"][c]
        m["logvalid"] = plan["logvalid"][c]
        in_maps.append(m)
    meta = dict(
        n=n, per=per, in_ch=in_ch, tiles=plan["tiles"], offs=plan["offs"],
        tt=plan["tt"], blocks=plan["blocks"], n_cores=n_cores,
    )
    return in_maps, meta


# ---------------------------------------------------------------- device


def build_program(meta, bench=False, debug_outs=False, ablate=None):
    import contextlib
    import concourse.bass as bass
    import concourse.tile as tile
    import concourse.mybir as mybir
    from concourse import bacc
    from concourse.masks import make_identity

    f32 = mybir.dt.float32
    bf16 = mybir.dt.bfloat16
    f32r = mybir.dt.float32r
    i32 = mybir.dt.int32

    n = meta["n"]
    per = meta["per"]
    in_ch = meta["in_ch"]
    tiles = meta["tiles"]
    offs = meta["offs"]
    tt = meta["tt"]
    blocks = meta["blocks"]
    n_cores = meta["n_cores"]
    perp = blocks * BLOCK  # xr tables padded to full blocks

    nt_full, nt_rem = divmod(per, 128)
    node_tiles = [(i * 128, 128) for i in range(nt_full)]
    if nt_rem:
        node_tiles.append((nt_full * 128, nt_rem))

    nc = bacc.Bacc("TRN2", target_bir_lowering=False, debug=False, num_devices=n_cores)

    # register a -1.0 const AP so scalar.add(x, -1.0) lowers on the ACT engine
    _cm1 = nc.alloc_sbuf_tensor("const-float32-neg1", [128, 1], f32)
    nc.gpsimd.memset(_cm1.ap(), -1.0)
    nc.const_aps.aps[(f32, -1.0)] = _cm1.ap()

    def din(name, shape, dt=f32):
        return nc.dram_tensor(name, shape, dt, kind="ExternalInput").ap()

    x_aug = din("x_aug", [in_ch + 1, per], f32r)
    w1l_aug = din("w1l_aug", [in_ch + 1, D], f32r)
    w1r_aug = din("w1r_aug", [in_ch + 1, D], f32r)
    w2l = din("w2l", [D, D], f32r)
    w2r = din("w2r", [D, D], f32r)
    b2l_row = din("b2l_row", [1, D], f32r)
    b2r_row = din("b2r_row", [1, D], f32r)
    wcls = din("wcls", [D, 64], f32r)
    bcls_row = din("bcls_row", [1, 64], f32r)
    att1_rep = din("att1_rep", [128, D])
    att2_rep = din("att2_rep", [128, D])
    att15_1_rep = din("att15_1_rep", [128, D])
    att15_2_rep = din("att15_2_rep", [128, D])
    bias1_rep = din("bias1_rep", [128, D])
    bias2_rep = din("bias2_rep", [128, D])
    esrc = din("esrc", [128, tt], i32)
    edst = din("edst", [128, tt], i32)
    logvalid = din("logvalid", [128, tt])
    kreps = din("kreps", [1, 8], i32) if bench else None

    out = nc.dram_tensor("out", [per, OUT_CH], f32, kind="ExternalOutput").ap()
    dbg_h1T = (
        nc.dram_tensor("dbg_h1T", [D, blocks * BLOCK], f32, kind="ExternalOutput").ap()
        if debug_outs
        else None
    )
    dbg_xl2 = (
        nc.dram_tensor("dbg_xl2", [per, W_TAB], f32, kind="ExternalOutput").ap()
        if debug_outs
        else None
    )
    dbg_h2T = (
        nc.dram_tensor("dbg_h2T", [D, blocks * BLOCK], f32, kind="ExternalOutput").ap()
        if debug_outs
        else None
    )

    with tile.TileContext(nc) as tc:
        with (
            tc.tile_pool(name="dram", bufs=1, space="DRAM") as dram,
            tc.tile_pool(name="consts", bufs=1) as consts,
            # ---- edge-phase pools, shared by both layers
            tc.tile_pool(name="eidx", bufs=2) as eidx,
            tc.tile_pool(name="eg", bufs=8) as eg,
            tc.tile_pool(name="esm", bufs=8) as esm,
            tc.tile_pool(name="eoh", bufs=4) as eoh,
            tc.tile_pool(name="ew", bufs=4) as ew,
            tc.tile_pool(name="exr", bufs=2) as exr,
            tc.tile_pool(name="eps", bufs=2, space="PSUM") as eps,
            tc.tile_pool(name="ebr", bufs=4, space="PSUM") as ebr,
            tc.tile_pool(name="etail", bufs=2) as etail,
        ):
            # ---------- persistent DRAM intermediates
            xl1_loc = dram.tile([per, W_TAB], f32r)
            xr1_loc = dram.tile([perp, D], bf16)
            xl1_full = dram.tile([n, W_TAB], f32r, addr_space="Shared")
            h1T = dram.tile([D, perp], f32r)
            xl2_loc = dram.tile([per, W_TAB], f32r)
            xr2_loc = dram.tile([perp, D], bf16)
            xl2_full = dram.tile([n, W_TAB], f32r, addr_space="Shared")
            h2T = dram.tile([D, perp], f32r)

            # ---------- constants in SBUF
            identity = consts.tile([128, 128], f32)
            make_identity(nc, identity[:])
            id_r_t = consts.tile([128, 128], f32r)
            nc.vector.tensor_copy(id_r_t[:], identity[:])
            id_r = id_r_t[:]
            id_bf_t = consts.tile([128, 128], bf16)
            nc.vector.tensor_copy(id_bf_t[:], identity[:])
            id_bf = id_bf_t[:]
            iota_f = consts.tile([128, 128], f32)
            iota_i = consts.tile([128, 128], i32)
            nc.gpsimd.iota(iota_i[:], pattern=[[1, 128]], base=0, channel_multiplier=0)
            nc.vector.tensor_copy(iota_f[:], iota_i[:])
            ones_f = consts.tile([1, 128], f32)
            nc.vector.memset(ones_f[:], 1.0)
            ones_sb = consts.tile([1, 128], f32r)
            nc.vector.tensor_copy(ones_sb[:], ones_f[:])

            if bench:
                kt = consts.tile([1, 8], i32)
                nc.sync.dma_start(kt[:], kreps[:])
                kregs = [nc.values_load(kt[0:1, j : j + 1]) for j in range(5)]

            def seg_loop(j):
                if bench:
                    return tc.For_i(0, kregs[j], 1)
                return contextlib.nullcontext()

            w1l_sb = consts.tile([in_ch + 1, D], f32r)
            nc.sync.dma_start(w1l_sb[:], w1l_aug[:])
            w1r_sb = consts.tile([in_ch + 1, D], f32r)
            nc.sync.dma_start(w1r_sb[:], w1r_aug[:])
            att1_sb = consts.tile([128, D], f32)
            nc.sync.dma_start(att1_sb[:], att1_rep[:])
            att2_sb = consts.tile([128, D], f32)
            nc.sync.dma_start(att2_sb[:], att2_rep[:])
            att15_1_sb = consts.tile([128, D], f32)
            nc.sync.dma_start(att15_1_sb[:], att15_1_rep[:])
            att15_2_sb = consts.tile([128, D], f32)
            nc.sync.dma_start(att15_2_sb[:], att15_2_rep[:])
            bias1_sb = consts.tile([128, D], f32)
            nc.sync.dma_start(bias1_sb[:], bias1_rep[:])
            bias2_sb = consts.tile([128, D], f32)
            nc.sync.dma_start(bias2_sb[:], bias2_rep[:])

            w2l_ch = []
            w2r_ch = []
            wcls_ch = []
            for k in range(4):
                t1 = consts.tile([128, D], f32r, name=f"w2l_{k}")
                nc.sync.dma_start(t1[:], w2l[k * 128 : (k + 1) * 128, :])
                w2l_ch.append(t1)
                t2 = consts.tile([128, D], f32r, name=f"w2r_{k}")
                nc.sync.dma_start(t2[:], w2r[k * 128 : (k + 1) * 128, :])
                w2r_ch.append(t2)
                t3 = consts.tile([128, 64], f32r, name=f"wcls_{k}")
                nc.sync.dma_start(t3[:], wcls[k * 128 : (k + 1) * 128, :])
                wcls_ch.append(t3)
            b2l_sb = consts.tile([1, D], f32r)
            nc.sync.dma_start(b2l_sb[:], b2l_row[:])
            b2r_sb = consts.tile([1, D], f32r)
            nc.sync.dma_start(b2r_sb[:], b2r_row[:])
            bcls_sb = consts.tile([1, 64], f32r)
            nc.sync.dma_start(bcls_sb[:], bcls_row[:])

            # zero-fill the xr tables' padded tail rows once
            if perp > per:
                zpad_f = consts.tile([128, D], f32)
                nc.vector.memset(zpad_f[:], 0.0)
                zpad = consts.tile([128, D], bf16)
                nc.vector.tensor_copy(zpad[:], zpad_f[:])
                nc.sync.dma_start(xr1_loc[per:perp, :], zpad[: perp - per, :])
                nc.sync.dma_start(xr2_loc[per:perp, :], zpad[: perp - per, :])

            xg_const = None
            if ablate == "nogather":
                xgc_f = consts.tile([128, W_TAB], f32)
                nc.vector.memset(xgc_f[:], 0.25)
                xg_const = consts.tile([128, W_TAB], f32r)
                nc.vector.tensor_copy(xg_const[:], xgc_f[:])

            # ================= stage 0: layer-1 dense transforms (local rows)
            with (
                tc.tile_pool(name="s0_in", bufs=3) as s0in,
                tc.tile_pool(name="s0_out", bufs=3) as s0out,
                seg_loop(0),
            ):
                for base, m in node_tiles:
                    lx = s0in.tile([in_ch + 1, 128], f32r, tag="lx")
                    nc.sync.dma_start(lx[:, :m], x_aug[:, base : base + m])
                    # l-branch: features + att-weighted row sums
                    psl = eps.tile([128, D], f32, space="PSUM", tag="out")
                    nc.tensor.matmul(
                        psl[:m, :], lhsT=lx[:, :m],
                        rhs=w1l_sb[:], start=True, stop=True,
                    )
                    sb = s0out.tile([128, W_TAB], f32r, tag="sb")
                    nc.scalar.copy(sb[:m, :D], psl[:m, :])
                    tmp = s0out.tile([128, D], f32, tag="tmp")
                    nc.vector.tensor_mul(tmp[:m, :], sb[:m, :D], att15_1_sb[:m, :])
                    with nc.allow_low_precision(reason="f32r is fp32-width"):
                        nc.vector.reduce_sum(
                            out=sb[:m, D:W_TAB],
                            in_=tmp[:m, :].rearrange("p (h c) -> p h c", h=HEADS),
                            axis=mybir.AxisListType.X,
                        )
                    nc.sync.dma_start(xl1_loc[base : base + m, :], sb[:m, :])
                    # r-branch: features only
                    psr = eps.tile([128, D], f32, space="PSUM", tag="out")
                    nc.tensor.matmul(
                        psr[:m, :], lhsT=lx[:, :m],
                        rhs=w1r_sb[:], start=True, stop=True,
                    )
                    sbr = s0out.tile([128, D], bf16, tag="sbr")
                    nc.scalar.copy(sbr[:m, :], psr[:m, :])
                    nc.sync.dma_start(xr1_loc[base : base + m, :], sbr[:m, :])

            nc.gpsimd.collective_compute(
                "AllGather",
                mybir.AluOpType.bypass,
                replica_groups=[list(range(n_cores))],
                ins=[xl1_loc.opt()],
                outs=[xl1_full.opt()],
            )

            # ================= edge phase (shared pools, both layers)
            def edge_phase(xl_full_ap, xr_loc_ap, att_sb, bias_sb, hT_ap, segj):
                with seg_loop(segj):
                    for b in range(blocks):
                        tb = tiles[b]
                        off = offs[b]
                        cbase = b * BLOCK
                        cols = min(BLOCK, per - cbase)
                        src_sb = eidx.tile([128, tb], i32, tag="src")
                        nc.sync.dma_start(src_sb[:], esrc[:, off : off + tb])
                        dst_sb = eidx.tile([128, tb], i32, tag="dst")
                        nc.sync.dma_start(dst_sb[:], edst[:, off : off + tb])
                        lv_sb = eidx.tile([128, tb], f32, tag="lv")
                        nc.sync.dma_start(lv_sb[:], logvalid[:, off : off + tb])
                        seg_f = eidx.tile([128, tb], f32, tag="seg")
                        nc.vector.tensor_copy(seg_f[:], dst_sb[:])
                        nc.vector.tensor_scalar_add(seg_f[:], seg_f[:], float(-cbase))

                        xr_blk = exr.tile([128, D], bf16, tag="xrb")
                        nc.sync.dma_start(xr_blk[:], xr_loc_ap[cbase : cbase + 128, :])

                        den_ps = eps.tile([128, 4], f32, space="PSUM", tag="den")
                        out_ps = eps.tile([128, D], f32, space="PSUM", tag="out")

                        for t in range(tb):
                            if ablate != "nogather":
                                xg = eg.tile([128, W_TAB], f32r, tag="xg")
                                nc.gpsimd.indirect_dma_start(
                                    out=xg[:],
                                    out_offset=None,
                                    in_=xl_full_ap,
                                    in_offset=bass.IndirectOffsetOnAxis(
                                        ap=src_sb[:, t : t + 1], axis=0
                                    ),
                                )
                            else:
                                xg = xg_const
                            if ablate == "onlygather":
                                continue
                            oh = eoh.tile([128, 128], bf16, tag="oh")
                            nc.vector.tensor_tensor(
                                out=oh[:],
                                in0=seg_f[:, t : t + 1].to_broadcast([128, 128]),
                                in1=iota_f[:],
                                op=mybir.AluOpType.is_equal,
                            )
                            if ablate != "nope":
                                ohT_ps = ebr.tile(
                                    [128, 128], bf16, space="PSUM", tag="xre"
                                )
                                nc.tensor.transpose(ohT_ps[:], oh[:], id_bf)
                                ohT = eoh.tile([128, 128], bf16, tag="ohT")
                                nc.scalar.copy(ohT[:], ohT_ps[:])
                                xr_e = ebr.tile([128, D], f32, space="PSUM", tag="xre")
                                nc.tensor.matmul(
                                    xr_e[:], lhsT=ohT[:],
                                    rhs=xr_blk[:], start=True, stop=True,
                                )
                                z = ew.tile([128, D], f32, tag="z")
                                nc.vector.tensor_add(z[:], xr_e[:], xg[:, :D])
                                ab_in = z
                            else:
                                ab_in = xg[:, :D]
                            if ablate != "nodve":
                                ab = ew.tile([128, D], f32, tag="ab")
                                nc.scalar.activation(
                                    ab[:], ab_in[:], mybir.ActivationFunctionType.Abs
                                )
                                nc.vector.tensor_mul(ab[:], ab[:], att_sb[:])
                                red = esm.tile([128, 4], f32, tag="red")
                                nc.vector.reduce_sum(
                                    out=red[:],
                                    in_=ab[:].rearrange("p (h c) -> p h c", h=HEADS),
                                    axis=mybir.AxisListType.X,
                                )
                                pre = esm.tile([128, 4], f32, tag="pre")
                                nc.vector.tensor_add(pre[:], xg[:, D:W_TAB], red[:])
                            else:
                                pre = esm.tile([128, 4], f32, tag="pre")
                                nc.vector.tensor_copy(pre[:], xg[:, D:W_TAB])
                            exps = esm.tile([128, 4], bf16, tag="exps")
                            nc.scalar.activation(
                                exps[:],
                                pre[:],
                                mybir.ActivationFunctionType.Exp,
                                bias=lv_sb[:, t : t + 1],
                                scale=0.4,
                            )
                            nc.tensor.matmul(
                                den_ps[:],
                                lhsT=oh[:],
                                rhs=exps[:],
                                start=(t == 0),
                                stop=(t == tb - 1),
                            )
                            if ablate != "nodve":
                                w = ew.tile([128, D], bf16, tag="w")
                                nc.vector.tensor_tensor(
                                    out=w[:].rearrange("p (h c) -> p h c", h=HEADS),
                                    in0=xg[:, :D].rearrange("p (h c) -> p h c", h=HEADS),
                                    in1=exps[:, :, None].to_broadcast(
                                        [128, HEADS, HID]
                                    ),
                                    op=mybir.AluOpType.mult,
                                )
                                w_ap = w[:]
                            else:
                                w_ap = xg[:, :D]
                            nc.tensor.matmul(
                                out_ps[:],
                                lhsT=oh[:],
                                rhs=w_ap,
                                start=(t == 0),
                                stop=(t == tb - 1),
                            )

                        # ---- block epilogue
                        if ablate == "onlygather":
                            continue
                        den_sb = esm.tile([128, 4], f32, tag="den_sb")
                        nc.vector.tensor_copy(den_sb[:], den_ps[:])
                        recip = esm.tile([128, 4], f32, tag="recip")
                        nc.vector.reciprocal(recip[:], den_sb[:])
                        h = etail.tile([128, D], f32, tag="h")
                        nc.vector.tensor_tensor(
                            out=h[:].rearrange("p (h c) -> p h c", h=HEADS),
                            in0=out_ps[:].rearrange("p (h c) -> p h c", h=HEADS),
                            in1=recip[:, :, None].to_broadcast([128, HEADS, HID]),
                            op=mybir.AluOpType.mult,
                        )
                        nc.vector.tensor_add(h[:], h[:], bias_sb[:])
                        # ELU: relu(x) + exp(min(x,0)) - 1
                        neg = etail.tile([128, D], f32, tag="neg")
                        nc.vector.tensor_scalar_min(neg[:], h[:], 0.0)
                        expn = etail.tile([128, D], f32, tag="expn")
                        nc.scalar.activation(
                            expn[:], neg[:], mybir.ActivationFunctionType.Exp
                        )
                        pos = etail.tile([128, D], f32, tag="pos")
                        nc.scalar.activation(
                            pos[:], h[:], mybir.ActivationFunctionType.Relu
                        )
                        hf = etail.tile([128, D], f32r, tag="hf")
                        nc.vector.tensor_add(hf[:], pos[:], expn[:])
                        nc.scalar.add(hf[:], hf[:], -1.0)
                        for q in range(4):
                            tp = ebr.tile([128, 128], f32r, space="PSUM", tag="xre")
                            nc.tensor.transpose(
                                tp[:], hf[:, q * 128 : (q + 1) * 128], id_r
                            )
                            tsb = etail.tile([128, 128], f32r, tag="tsb")
                            nc.scalar.copy(tsb[:], tp[:])
                            nc.sync.dma_start(
                                hT_ap[q * 128 : (q + 1) * 128, cbase : cbase + cols],
                                tsb[:, :cols],
                            )

            edge_phase(xl1_full.opt(), xr1_loc.opt(), att1_sb, bias1_sb, h1T.opt(), 1)

            # ================= stage 2: layer-2 dense transforms from h1T
            with (
                tc.tile_pool(name="s2_in", bufs=4) as s2in,
                tc.tile_pool(name="s2_out", bufs=3) as s2out,
                seg_loop(2),
            ):
                for base, m in node_tiles:
                    hts = []
                    for k in range(4):
                        ht = s2in.tile([128, 128], f32r, tag=f"ht{k}")
                        nc.sync.dma_start(
                            ht[:, :m], h1T[k * 128 : (k + 1) * 128, base : base + m]
                        )
                        hts.append(ht)
                    # l-branch
                    psl = eps.tile([128, D], f32, space="PSUM", tag="out")
                    for k in range(4):
                        nc.tensor.matmul(
                            psl[:m, :], lhsT=hts[k][:, :m],
                            rhs=w2l_ch[k][:],
                            start=(k == 0), stop=False,
                        )
                    nc.tensor.matmul(
                        psl[:m, :], lhsT=ones_sb[:, :m],
                        rhs=b2l_sb[:], start=False, stop=True,
                    )
                    sb = s2out.tile([128, W_TAB], f32r, tag="sb")
                    nc.scalar.copy(sb[:m, :D], psl[:m, :])
                    tmp = s2out.tile([128, D], f32, tag="tmp")
                    nc.vector.tensor_mul(tmp[:m, :], sb[:m, :D], att15_2_sb[:m, :])
                    with nc.allow_low_precision(reason="f32r is fp32-width"):
                        nc.vector.reduce_sum(
                            out=sb[:m, D:W_TAB],
                            in_=tmp[:m, :].rearrange("p (h c) -> p h c", h=HEADS),
                            axis=mybir.AxisListType.X,
                        )
                    nc.sync.dma_start(xl2_loc[base : base + m, :], sb[:m, :])
                    # r-branch
                    psr = eps.tile([128, D], f32, space="PSUM", tag="out")
                    for k in range(4):
                        nc.tensor.matmul(
                            psr[:m, :], lhsT=hts[k][:, :m],
                            rhs=w2r_ch[k][:],
                            start=(k == 0), stop=False,
                        )
                    nc.tensor.matmul(
                        psr[:m, :], lhsT=ones_sb[:, :m],
                        rhs=b2r_sb[:], start=False, stop=True,
                    )
                    sbr = s2out.tile([128, D], bf16, tag="sbr")
                    nc.scalar.copy(sbr[:m, :], psr[:m, :])
                    nc.sync.dma_start(xr2_loc[base : base + m, :], sbr[:m, :])

            nc.gpsimd.collective_compute(
                "AllGather",
                mybir.AluOpType.bypass,
                replica_groups=[list(range(n_cores))],
                ins=[xl2_loc.opt()],
                outs=[xl2_full.opt()],
            )

            edge_phase(xl2_full.opt(), xr2_loc.opt(), att2_sb, bias2_sb, h2T.opt(), 3)

            # ================= classifier
            with (
                tc.tile_pool(name="c_in", bufs=4) as cin,
                tc.tile_pool(name="c_out", bufs=3) as cout,
                seg_loop(4),
            ):
                for base, m in node_tiles:
                    hts = []
                    for k in range(4):
                        ht = cin.tile([128, 128], f32r, tag=f"cht{k}")
                        nc.sync.dma_start(
                            ht[:, :m], h2T[k * 128 : (k + 1) * 128, base : base + m]
                        )
                        hts.append(ht)
                    ps = eps.tile([128, 64], f32, space="PSUM", tag="out")
                    for k in range(4):
                        nc.tensor.matmul(
                            ps[:m, :], lhsT=hts[k][:, :m],
                            rhs=wcls_ch[k][:],
                            start=(k == 0), stop=False,
                        )
                    nc.tensor.matmul(
                        ps[:m, :], lhsT=ones_sb[:, :m],
                        rhs=bcls_sb[:], start=False, stop=True,
                    )
                    sb = cout.tile([128, 64], f32, tag="sb")
                    nc.scalar.copy(sb[:m, :], ps[:m, :])
                    nc.sync.dma_start(out[base : base + m, :], sb[:m, :OUT_CH])

    nc.compile()
    return nc


# ---------------------------------------------------------------- entry

_CACHE = {}


def kernel(**inputs):
    from concourse.bass_utils import run_bass_kernel_spmd

    in_maps, meta = preprocess(**inputs)
    key = (meta["tt"], tuple(meta["tiles"]))
    if key not in _CACHE:
        _CACHE[key] = build_program(meta)
    nc = _CACHE[key]
    res = run_bass_kernel_spmd(nc, in_maps, list(range(meta["n_cores"])))
    outs = [res.results[c]["out"] for c in range(meta["n_cores"])]
    return np.concatenate(outs, axis=0)


# ---------------------------------------------------------------- numpy model
# (host-side mirror of the device math, for validation in test.py)


def numpy_model(x, edge_index, w1_l, b1_l, w1_r, b1_r, att1, bias1,
                w2_l, b2_l, w2_r, b2_r, att2, bias2, w_cls, b_cls):
    x = np.asarray(x, np.float32)
    n = x.shape[0]
    ei = np.asarray(edge_index).astype(np.int64)
    loops = np.arange(n, dtype=np.int64)
    src = np.concatenate([ei[0], loops])
    dst = np.concatenate([ei[1], loops])

    def layer(h, wl, bl, wr, br, att, bias):
        att_flat = np.asarray(att, np.float32).reshape(-1)
        xl = (h @ np.asarray(wl, np.float32) + np.asarray(bl, np.float32)).astype(np.float32)
        xr = (h @ np.asarray(wr, np.float32) + np.asarray(br, np.float32)).astype(np.float32)
        adl = (xl * att_flat).reshape(n, HEADS, HID).sum(axis=2)
        z = xl[src] + xr[dst]
        abssum = (np.abs(z) * att_flat).reshape(-1, HEADS, HID).sum(axis=2)
        # adr[dst] is constant per softmax segment -> dropped (shift invariance)
        score = 0.4 * (abssum + 1.5 * adl[src])
        ex = np.exp(score)
        den = np.zeros((n, HEADS), np.float32)
        np.add.at(den, dst, ex)
        unnorm = np.zeros((n, HEADS, HID), np.float32)
        np.add.at(unnorm, dst, xl[src].reshape(-1, HEADS, HID) * ex[:, :, None])
        out = unnorm / den[:, :, None]
        out = out.reshape(n, D) + np.asarray(bias, np.float32)
        return np.where(out > 0, out, np.exp(np.minimum(out, 0)) - 1).astype(np.float32)

    h = layer(x, w1_l, b1_l, w1_r, b1_r, att1, bias1)
    h = layer(h, w2_l, b2_l, w2_r, b2_r, att2, bias2)
    return (h @ np.asarray(w_cls, np.float32) + np.asarray(b_cls, np.float32)).astype(
        np.float32
    )
